# revision 32
# baseline (speedup 1.0000x reference)
"""Trainium2 Bass kernel for nn_DoubleNet (topk_masking).

Computation (see reference):
  5 hidden layers: h = relu(h @ (m1_l - m2_l).T + 2*b_l)   [8192, 4096]
  output layer:    h = relu(h @ (m1_o - m2_o).T + 2*b_o)   [8192, 2]
  final:           y = h @ w_last.T                        [8192, 1]
where m1/m2 are top-50% masks of |scores| (exact argsort tie semantics).

Strategy:
  - Masks are x-independent: computed exactly on host with an O(n)
    partition + stable tie-fix that matches jnp.argsort(stable) bit-exactly.
  - Data-parallel: batch 8192 split as 1024 rows per NeuronCore (8 cores).
  - Weights w = m1 - m2 in {-1, 0, +1} are exact in fp16.
  - Per core, all 1024 activation cols stay resident in SBUF through all
    layers; weight slabs stream from HBM (168MB/core), triple-buffered,
    each slab reused for both 512-col batch halves.

MODE:
  "strass" (default): one-level Strassen over every 4096x4096 layer (W
          split 2x2, h split k-half x batch-half): 7 products instead of
          8 -> 1792 MMs/layer vs 2048, beating the classical fp16 PE
          "roofline" by ~12%. Weight combos (entries {-2..2}) are exact
          in fp16 and host-precomputed; activation combos are DVE adds
          hidden under PE work; eager-drain C-recombination keeps PSUM
          pressure at ~1 bank. ~3.6e-3 rel err (gate is 2e-2), ~1.97ms.
  "fp16s": classical fp16 single pass at the PE streaming floor
          (~2.24ms): 1 rhs column/cycle at N=512 regardless of
          dtype/perf-mode (measured: fp16=bf16=fp8DR ~216ns/MM warm);
          pure-fp8 DoubleRow would be 2x but fails the 2e-2 gate (6e-2),
          and any hi+lo fp8 correction costs exactly fp16. ~1.8e-3 err.
  "fp16m": fp16s predecessor (merged batch, no edge tuning).
  "split2": hi/lo bf16, 2 matmul passes, ~2.5e-5 rel err (2x slower).
  "fp16": like fp16m but 2 batch chunks of 512 (2x weight DMA).
  "hybrid": fp16 + fp8 DoubleRow lo pass, ~4.6e-5 rel err (1.5x slower).
"""

import sys

for _p in ("/opt/trn_rl_repo", "/root/.axon_site/_ro/trn_rl_repo"):
    if _p not in sys.path:
        sys.path.insert(0, _p)

import numpy as np
import ml_dtypes

import concourse.mybir as mybir
import concourse.tile as tile
from concourse import bacc
from concourse.bass_utils import run_bass_kernel_spmd

BF = ml_dtypes.bfloat16
F16 = np.float16
BF16 = mybir.dt.bfloat16
FP16 = mybir.dt.float16
F32 = mybir.dt.float32

P = 128          # partitions
N_CORES = 8
B = 8192         # total batch
D = 4096         # width
L = 5            # hidden layers
KEEP = 0.5
NT = D // P      # 32 d/ko tiles
BC = B // N_CORES  # 1024 batch rows per core
NB = 512         # matmul free dim (one PSUM bank of fp32)
CH = BC // NB    # 2 chunks per core

MODE = "strass"  # "split2" | "fp16" | "hybrid" | "fp16m" | "fp16s" | "strass"
FP16_SHIFT = 6   # fp16/hybrid: h_l is carried scaled by 2^(-FP16_SHIFT*l)
LOSC = 8192.0
WARMUP_MMS = 16   # fp16m: PE warmup matmuls during the x load
WARMUP_MMS_S = 14  # fp16s: warmup sized to end ~ when piece0+slab0 land
KO0_FILLERS = 8    # fp16s: discard MMs inside (l0,h0,ko0), paced by x pieces
POST_KO0_FILLERS = 8  # fp16s: discard MMs between ko0 and ko1 (slab1 DMA gap)
STRASS_WARMUP = 12    # strass: PE warmup MMs during the x/slab load
# strass: filler MMs before (ko, exec-slot) of layer 0, bridging DMA waits
STRASS_FILLERS = {(0, 2): 14, (0, 3): 18, (0, 4): 6, (0, 5): 6, (0, 6): 6,
                  (1, 0): 4}
# LOSC: hybrid mode: lo residual is stored as fp8 scaled by 2^13

_BUILD_CACHE = {}


def _subnet_mask(scores: np.ndarray) -> np.ndarray:
    """Exact replica of reference.get_subnet(|scores|) forward value.

    Zero the j smallest |scores| (ties at the threshold broken by flat
    index order, matching stable argsort), one elsewhere.
    """
    flat = np.abs(scores.astype(np.float32, copy=False)).ravel()
    n = flat.size
    j = int((1.0 - KEEP) * n)
    if j == 0:
        return np.ones(scores.shape, np.float32)
    thr = np.partition(flat, j - 1)[j - 1]
    mask = (flat > thr).astype(np.float32)
    c_lt = int((flat < thr).sum())
    idx_eq = np.flatnonzero(flat == thr)
    n_zero_eq = j - c_lt
    assert 0 <= n_zero_eq <= idx_eq.size
    mask[idx_eq[n_zero_eq:]] = 1.0
    return mask.reshape(scores.shape)


def _build_split2():
    nc = bacc.Bacc("TRN2", target_bir_lowering=False, debug=False)
    xt_d = nc.dram_tensor("xt", [CH, 2, P, NT * NB], BF16, kind="ExternalInput").ap()
    wt_d = nc.dram_tensor("wt", [L, NT, P, NT * P], BF16, kind="ExternalInput").ap()
    bias_d = nc.dram_tensor("biasd", [P, L * NT], F32, kind="ExternalInput").ap()
    wo_d = nc.dram_tensor("wo", [P, NT * 2], BF16, kind="ExternalInput").ap()
    z_d = nc.dram_tensor("z", [2, BC], F32, kind="ExternalOutput").ap()

    with tile.TileContext(nc) as tc:
        with (
            tc.tile_pool(name="acts", bufs=1) as acts,
            tc.tile_pool(name="wpool", bufs=3) as wpool,
            tc.tile_pool(name="tmp", bufs=3) as tpool,
            tc.tile_pool(name="const", bufs=1) as cpool,
            tc.tile_pool(name="psum", bufs=2, space="PSUM") as ppool,
            tc.tile_pool(name="wps", bufs=1, space="PSUM") as wppool,
            tc.tile_pool(name="zpsum", bufs=2, space="PSUM") as zppool,
            tc.tile_pool(name="zsb", bufs=2) as zsbpool,
        ):
            A_hi = acts.tile([P, NT * NB], BF16, name="A_hi")
            A_lo = acts.tile([P, NT * NB], BF16, name="A_lo")
            B_hi = acts.tile([P, NT * NB], BF16, name="B_hi")
            B_lo = acts.tile([P, NT * NB], BF16, name="B_lo")
            bias_sb = cpool.tile([P, L * NT], F32, name="bias_sb")
            wo_sb = cpool.tile([P, NT * 2], BF16, name="wo_sb")
            nc.sync.dma_start(bias_sb[:], bias_d[:])
            nc.sync.dma_start(wo_sb[:], wo_d[:])

            for c in range(CH):
                nc.sync.dma_start(A_hi[:], xt_d[c, 0])
                nc.sync.dma_start(A_lo[:], xt_d[c, 1])
                for l in range(L):
                    ain_hi, ain_lo = (A_hi, A_lo) if l % 2 == 0 else (B_hi, B_lo)
                    aout_hi, aout_lo = (B_hi, B_lo) if l % 2 == 0 else (A_hi, A_lo)
                    for ko in range(NT):
                        slab = wpool.tile([P, NT * P], BF16, name="wslab")
                        nc.sync.dma_start(slab[:], wt_d[l, ko])
                        pt = ppool.tile([P, NB], F32, name="pt")
                        for d in range(NT):
                            lhsT = slab[:, d * P : (d + 1) * P]
                            nc.tensor.matmul(
                                pt[:], lhsT, ain_hi[:, d * NB : (d + 1) * NB],
                                start=(d == 0), stop=False,
                            )
                            nc.tensor.matmul(
                                pt[:], lhsT, ain_lo[:, d * NB : (d + 1) * NB],
                                start=False, stop=(d == NT - 1),
                            )
                        tmp = tpool.tile([P, NB], F32, name="tmp")
                        nc.scalar.activation(
                            tmp[:], pt[:], mybir.ActivationFunctionType.Relu,
                            bias=bias_sb[:, l * NT + ko : l * NT + ko + 1], scale=1.0,
                        )
                        nc.vector.tensor_copy(
                            aout_hi[:, ko * NB : (ko + 1) * NB], tmp[:]
                        )
                        nc.vector.tensor_sub(
                            aout_lo[:, ko * NB : (ko + 1) * NB],
                            tmp[:],
                            aout_hi[:, ko * NB : (ko + 1) * NB],
                        )

                # output layer: z[2, NB] = w_o @ h5 (pre-bias, pre-relu)
                hout_hi, hout_lo = (A_hi, A_lo) if L % 2 == 0 else (B_hi, B_lo)
                zp = zppool.tile([2, NB], F32, name="zp")
                for t in range(NT):
                    lhsT = wo_sb[:, t * 2 : (t + 1) * 2]
                    nc.tensor.matmul(
                        zp[:], lhsT, hout_hi[:, t * NB : (t + 1) * NB],
                        start=(t == 0), stop=False,
                    )
                    nc.tensor.matmul(
                        zp[:], lhsT, hout_lo[:, t * NB : (t + 1) * NB],
                        start=False, stop=(t == NT - 1),
                    )
                zs = zsbpool.tile([2, NB], F32, name="zs")
                nc.vector.tensor_copy(zs[:], zp[:])
                nc.sync.dma_start(z_d[:, c * NB : (c + 1) * NB], zs[:])

    nc.compile()
    return nc


def _build_fp16():
    nc = bacc.Bacc("TRN2", target_bir_lowering=False, debug=False)
    xt_d = nc.dram_tensor("xt", [CH, P, NT * NB], FP16, kind="ExternalInput").ap()
    wt_d = nc.dram_tensor("wt", [L, NT, P, NT * P], FP16, kind="ExternalInput").ap()
    bias_d = nc.dram_tensor("biasd", [P, L * NT], F32, kind="ExternalInput").ap()
    wo_d = nc.dram_tensor("wo", [P, NT * 2], FP16, kind="ExternalInput").ap()
    z_d = nc.dram_tensor("z", [2, BC], F32, kind="ExternalOutput").ap()
    sc = float(2.0 ** (-FP16_SHIFT))

    with tile.TileContext(nc) as tc:
        with (
            tc.tile_pool(name="acts", bufs=1) as acts,
            tc.tile_pool(name="wpool", bufs=3) as wpool,
            tc.tile_pool(name="const", bufs=1) as cpool,
            tc.tile_pool(name="psum", bufs=2, space="PSUM") as ppool,
            tc.tile_pool(name="wps", bufs=1, space="PSUM") as wppool,
            tc.tile_pool(name="zpsum", bufs=2, space="PSUM") as zppool,
            tc.tile_pool(name="zsb", bufs=2) as zsbpool,
        ):
            A = acts.tile([P, NT * NB], FP16, name="A")
            Bt = acts.tile([P, NT * NB], FP16, name="Bt")
            bias_sb = cpool.tile([P, L * NT], F32, name="bias_sb")
            wo_sb = cpool.tile([P, NT * 2], FP16, name="wo_sb")
            nc.sync.dma_start(bias_sb[:], bias_d[:])
            nc.sync.dma_start(wo_sb[:], wo_d[:])

            for c in range(CH):
                nc.sync.dma_start(A[:], xt_d[c])
                for l in range(L):
                    ain = A if l % 2 == 0 else Bt
                    aout = Bt if l % 2 == 0 else A
                    for ko in range(NT):
                        slab = wpool.tile([P, NT * P], FP16, name="wslab")
                        nc.sync.dma_start(slab[:], wt_d[l, ko])
                        pt = ppool.tile([P, NB], F32, name="pt")
                        for d in range(NT):
                            nc.tensor.matmul(
                                pt[:], slab[:, d * P : (d + 1) * P],
                                ain[:, d * NB : (d + 1) * NB],
                                start=(d == 0), stop=(d == NT - 1),
                            )
                        # g_{l+1} = relu(2^-S * psum + 2*b*2^(-S(l+1))), fp16 out
                        nc.scalar.activation(
                            aout[:, ko * NB : (ko + 1) * NB], pt[:],
                            mybir.ActivationFunctionType.Relu,
                            bias=bias_sb[:, l * NT + ko : l * NT + ko + 1], scale=sc,
                        )

                hout = A if L % 2 == 0 else Bt
                zp = zppool.tile([2, NB], F32, name="zp")
                for t in range(NT):
                    nc.tensor.matmul(
                        zp[:], wo_sb[:, t * 2 : (t + 1) * 2],
                        hout[:, t * NB : (t + 1) * NB],
                        start=(t == 0), stop=(t == NT - 1),
                    )
                zs = zsbpool.tile([2, NB], F32, name="zs")
                nc.vector.tensor_copy(zs[:], zp[:])
                nc.sync.dma_start(z_d[:, c * NB : (c + 1) * NB], zs[:])

    nc.compile()
    return nc


def _build_fp16m():
    """fp16 single-pass, merged batch (1024 cols/core in one sweep).

    vs _build_fp16: weights are loaded once per (l, ko) slab and used for
    both 512-col batch halves (halves HBM weight traffic to 168MB/core and
    removes the inter-chunk PE gap); x is DMA'd in 4 pieces so layer 0 can
    start before the full 8MB lands.
    """
    nc = bacc.Bacc("TRN2", target_bir_lowering=False, debug=False)
    xt_d = nc.dram_tensor("xt", [P, NT * BC], FP16, kind="ExternalInput").ap()
    wt_d = nc.dram_tensor("wt", [L, NT, P, NT * P], FP16, kind="ExternalInput").ap()
    bias_d = nc.dram_tensor("biasd", [P, L * NT], F32, kind="ExternalInput").ap()
    wo_d = nc.dram_tensor("wo", [P, NT * 2], FP16, kind="ExternalInput").ap()
    sel_d = nc.dram_tensor("sel", [P, 2], FP16, kind="ExternalInput").ap()
    z_d = nc.dram_tensor("z", [2, BC], F32, kind="ExternalOutput").ap()
    sc = float(2.0 ** (-FP16_SHIFT))
    XP = 8  # x DMA pieces

    with tile.TileContext(nc) as tc:
        with (
            tc.tile_pool(name="acts", bufs=1) as acts,
            tc.tile_pool(name="wpool", bufs=3) as wpool,
            tc.tile_pool(name="const", bufs=1) as cpool,
            tc.tile_pool(name="warm", bufs=1) as warmpool,
            tc.tile_pool(name="psum", bufs=2, space="PSUM") as ppool,
            tc.tile_pool(name="wps", bufs=1, space="PSUM") as wppool,
            tc.tile_pool(name="zpsum", bufs=2, space="PSUM") as zppool,
            tc.tile_pool(name="zsb", bufs=2) as zsbpool,
        ):
            A = acts.tile([P, NT * BC], FP16, name="A")
            Bt = acts.tile([P, NT * BC], FP16, name="Bt")
            bias_sb = cpool.tile([P, L * NT], F32, name="bias_sb")
            wo_sb = cpool.tile([P, NT * 2], FP16, name="wo_sb")
            xpc = NT * BC // XP
            # DMA issue order matters: the rings drain in order, so the
            # first weight slab + consts must not queue behind all 8MB of
            # x (that cost a 15us PE stall). piece0 -> slab0 -> consts ->
            # remaining x pieces.
            nc.sync.dma_start(A[:, 0:xpc], xt_d[:, 0:xpc])
            slab0 = wpool.tile([P, NT * P], FP16, name="wslab")
            nc.sync.dma_start(slab0[:], wt_d[0, 0])
            nc.sync.dma_start(bias_sb[:], bias_d[:])
            nc.sync.dma_start(wo_sb[:], wo_d[:])
            for pc in range(1, XP):
                nc.sync.dma_start(
                    A[:, pc * xpc : (pc + 1) * xpc],
                    xt_d[:, pc * xpc : (pc + 1) * xpc],
                )

            # PE warmup while x/weights stream in: keeps the HAM activity
            # window busy (and the PE instruction queue deep) so the first
            # real matmuls run at 2.4GHz with no sync micro-gaps. Sized to
            # roughly bridge the ~22us x-load.
            wt_warm = warmpool.tile([P, P + NB], FP16, name="wt_warm")
            nc.vector.memset(wt_warm[:], 0.0)
            wp = wppool.tile([P, NB], F32, name="wp")
            for i in range(WARMUP_MMS):
                nc.tensor.matmul(wp[:], wt_warm[:, 0:P], wt_warm[:, P:],
                                 start=(i == 0), stop=(i == WARMUP_MMS - 1))

            for l in range(L):
                ain = A if l % 2 == 0 else Bt
                aout = Bt if l % 2 == 0 else A
                for ko in range(NT):
                    if l == 0 and ko == 0:
                        slab = slab0
                    else:
                        slab = wpool.tile([P, NT * P], FP16, name="wslab")
                        nc.sync.dma_start(slab[:], wt_d[l, ko])
                    # both batch halves interleaved per k-tile (one 2-bank
                    # psum tile; each MM's out slice stays within a bank):
                    # layer 0 then paces both halves with the arriving x
                    # pieces instead of re-running half after the x window
                    pt = ppool.tile([P, 2 * NB], F32, name="pt")
                    for d in range(NT):
                        for h in range(2):
                            nc.tensor.matmul(
                                pt[:, h * NB : (h + 1) * NB],
                                slab[:, d * P : (d + 1) * P],
                                ain[:, d * BC + h * NB : d * BC + (h + 1) * NB],
                                start=(d == 0), stop=(d == NT - 1),
                            )
                    for h in range(2):
                        nc.scalar.activation(
                            aout[:, ko * BC + h * NB : ko * BC + (h + 1) * NB],
                            pt[:, h * NB : (h + 1) * NB],
                            mybir.ActivationFunctionType.Relu,
                            bias=bias_sb[:, l * NT + ko : l * NT + ko + 1], scale=sc,
                        )

            # Output layer, col-group packed: the [128k, 2out] matmuls use
            # only 2 of 128 PE columns, so run 4 k-tiles concurrently in
            # col groups {0,32,64,96} (4 partial z pairs), then reduce the
            # 4 partials across partitions with a tiny selection matmul.
            hout = A if L % 2 == 0 else Bt
            zevs = []
            for h in range(2):
                zev = cpool.tile([P, NB], FP16, name=f"zev{h}")
                nc.vector.memset(zev[:], 0.0)
                zevs.append(zev)
            sel = cpool.tile([P, 2], FP16, name="sel")
            nc.sync.dma_start(sel[:], sel_d[:])
            # both packed groups back-to-back on the PE; the per-half DVE
            # evictions overlap the other half's matmuls, combines at the end
            for h in range(2):
                zp4 = zppool.tile([P, NB], F32, name="zp4")
                for t in range(NT):
                    j = t % 4
                    nc.tensor.matmul(
                        zp4[32 * j : 32 * j + 2, :], wo_sb[:, t * 2 : (t + 1) * 2],
                        hout[:, t * BC + h * NB : t * BC + (h + 1) * NB],
                        start=(t < 4), stop=(t >= NT - 4),
                        tile_position=(0, 32 * j),
                    )
                for j in range(4):
                    nc.vector.tensor_copy(
                        zevs[h][32 * j : 32 * j + 2, :], zp4[32 * j : 32 * j + 2, :]
                    )
            for h in range(2):
                zpf = wppool.tile([2, NB], F32, name="zpf")
                nc.tensor.matmul(zpf[:], sel[:], zevs[h][:], start=True, stop=True)
                zs = zsbpool.tile([2, NB], F32, name="zs")
                nc.vector.tensor_copy(zs[:], zpf[:])
                nc.sync.dma_start(z_d[:, h * NB : (h + 1) * NB], zs[:])

    nc.compile()
    return nc


def _build_fp16s():
    """fp16 single-pass like fp16m, tuned at the edges (trace-driven).

    vs _build_fp16m:
      - Startup is DMA-BW-bound (~420GB/s aggregate): layer 0 runs as two
        512-col half-sweeps (slab reloaded per half), and the first THREE
        ko groups of the h0 sweep are interleaved d-wise with staggered
        offsets - they all read the same arriving x pieces, so the PE gets
        3x the work per landed x byte and is never starved while x-half0
        (4MB) streams in.  x is stored half-major [2, P, NT*NB]; slab0 is
        chunked so the very first MM only needs ~0.26MB.  bias/wo go on
        the scalar engine's DMA queue (idle at startup).  Discardable
        filler MMs cover the residual DMA-behind instants.
      - Warmup lhsT memset on gpsimd (free ~1us earlier than DVE).
      - Output layer: per-half col-group-packed partial bursts are
        interleaved into the last hidden layer's ko loop (only the last
        burst + eviction remain after the final matmul); each [128,512]
        f32 PSUM tile is evicted whole (scalar h0 / vector h1, runs
        concurrently) and DMA'd raw - host sums partitions {32j,32j+1}.
    """
    nc = bacc.Bacc("TRN2", target_bir_lowering=False, debug=False)
    xt_d = nc.dram_tensor("xt", [2, P, NT * NB], FP16, kind="ExternalInput").ap()
    wt_d = nc.dram_tensor("wt", [L, NT, P, NT * P], FP16, kind="ExternalInput").ap()
    bias_d = nc.dram_tensor("biasd", [P, L * NT], F32, kind="ExternalInput").ap()
    wo_d = nc.dram_tensor("wo", [P, NT * 2], FP16, kind="ExternalInput").ap()
    z_d = nc.dram_tensor("z", [2, P, NB], F32, kind="ExternalOutput").ap()
    sc = float(2.0 ** (-FP16_SHIFT))
    XP = 8           # x DMA pieces per half (0.5MB each = 4 d-tiles)
    HNB = NT * NB    # columns per half in the layer-0 x layout

    with tile.TileContext(nc) as tc:
        with (
            tc.tile_pool(name="acts", bufs=1) as acts,
            tc.tile_pool(name="wpool", bufs=3) as wpool,
            tc.tile_pool(name="const", bufs=1) as cpool,
            tc.tile_pool(name="warm", bufs=1) as warmpool,
            tc.tile_pool(name="psum", bufs=2, space="PSUM") as ppool,
            tc.tile_pool(name="wps", bufs=1, space="PSUM") as wppool,
            tc.tile_pool(name="zpsum", bufs=1, space="PSUM") as zpool,
            tc.tile_pool(name="zsb", bufs=1) as zsbpool,
        ):
            A = acts.tile([P, NT * BC], FP16, name="A")
            Bt = acts.tile([P, NT * BC], FP16, name="Bt")
            bias_sb = cpool.tile([P, L * NT], F32, name="bias_sb")
            wo_sb = cpool.tile([P, NT * 2], FP16, name="wo_sb")
            xpc = HNB // XP
            # consts go on the scalar engine's hw DMA queue (idle at start);
            # sync-queue order: x-half0 pieces first (ko0 paces behind them),
            # slab0 right after piece0, then slabs 1-2; x-half1 pieces
            # interleave into the h0 ko-loop below.
            nc.scalar.dma_start(bias_sb[:], bias_d[:])
            nc.scalar.dma_start(wo_sb[:], wo_d[:])
            nc.sync.dma_start(A[:, 0:xpc], xt_d[0, :, 0:xpc])
            slab_pre = []
            s = wpool.tile([P, NT * P], FP16, name="wslab")
            nc.sync.dma_start(s[:], wt_d[0, 0])
            slab_pre.append(s)
            for pc in range(1, XP):
                nc.sync.dma_start(
                    A[:, pc * xpc : (pc + 1) * xpc], xt_d[0, :, pc * xpc : (pc + 1) * xpc]
                )
            for ko in (1, 2):
                s = wpool.tile([P, NT * P], FP16, name="wslab")
                nc.sync.dma_start(s[:], wt_d[0, ko])
                slab_pre.append(s)

            # PE warmup during the x/slab load (HAM ramp + queue fill).
            wt_warm = warmpool.tile([P, P + NB], FP16, name="wt_warm")
            nc.gpsimd.memset(wt_warm[:], 0.0)
            wp = wppool.tile([P, NB], F32, name="wp")
            for i in range(WARMUP_MMS_S):
                nc.tensor.matmul(wp[:], wt_warm[:, 0:P], wt_warm[:, P:],
                                 start=(i == 0), stop=(i == WARMUP_MMS_S - 1))

            def filler(j):
                # discardable MM whose rhs is an already-arrived x piece, so
                # the scheduler can't run it before that piece's DMA.
                nc.tensor.matmul(wp[:], wt_warm[:, 0:P],
                                 A[:, j * xpc : j * xpc + NB],
                                 start=True, stop=True)

            # Layer 0: two half-sweeps (slab reloaded per half).
            for h in range(2):
                for ko in range(NT):
                    if h == 0 and ko < 3:
                        slab = slab_pre[ko]
                    else:
                        slab = wpool.tile([P, NT * P], FP16, name="wslab")
                        nc.sync.dma_start(slab[:], wt_d[0, ko])
                    if h == 0 and 3 <= ko <= 10:
                        # stream an x-half1 piece between slab DMAs
                        pc = ko - 3
                        nc.sync.dma_start(
                            A[:, HNB + pc * xpc : HNB + (pc + 1) * xpc],
                            xt_d[1, :, pc * xpc : (pc + 1) * xpc],
                        )
                    pt = ppool.tile([P, NB], F32, name="pt")
                    for d in range(NT):
                        nc.tensor.matmul(
                            pt[:], slab[:, d * P : (d + 1) * P],
                            A[:, h * HNB + d * NB : h * HNB + (d + 1) * NB],
                            start=(d == 0), stop=(d == NT - 1),
                        )
                        if h == 0 and ko == 0 and d % 4 == 3 and d // 4 < KO0_FILLERS:
                            filler(d // 4)
                    if h == 0 and ko == 0:
                        for g in range(POST_KO0_FILLERS):
                            filler(g % XP)
                    nc.scalar.activation(
                        Bt[:, ko * BC + h * NB : ko * BC + (h + 1) * NB], pt[:],
                        mybir.ActivationFunctionType.Relu,
                        bias=bias_sb[:, ko : ko + 1], scale=sc,
                    )

            # Layers 1..4 (merged 1024-col sweeps, slab reused for both halves)
            for l in range(1, L):
                ain = Bt if l % 2 == 1 else A
                aout = A if l % 2 == 1 else Bt
                for ko in range(NT):
                    slab = wpool.tile([P, NT * P], FP16, name="wslab")
                    nc.sync.dma_start(slab[:], wt_d[l, ko])
                    pt = ppool.tile([P, 2 * NB], F32, name="pt")
                    for d in range(NT):
                        for h in range(2):
                            nc.tensor.matmul(
                                pt[:, h * NB : (h + 1) * NB],
                                slab[:, d * P : (d + 1) * P],
                                ain[:, d * BC + h * NB : d * BC + (h + 1) * NB],
                                start=(d == 0), stop=(d == NT - 1),
                            )
                    for h in range(2):
                        nc.scalar.activation(
                            aout[:, ko * BC + h * NB : ko * BC + (h + 1) * NB],
                            pt[:, h * NB : (h + 1) * NB],
                            mybir.ActivationFunctionType.Relu,
                            bias=bias_sb[:, l * NT + ko : l * NT + ko + 1], scale=sc,
                        )

            # Output layer: col-group packed partials per half (the burst
            # stream overlaps ko31's ACT drain); evict each [128,512] f32
            # PSUM tile whole (scalar h0 / vector h1, concurrently) and DMA
            # raw - host sums partitions {32j,32j+1}.
            hout = A if L % 2 == 0 else Bt
            zps = []
            for h in range(2):
                zp = zpool.tile([P, NB], F32, name=f"zp{h}")
                zps.append(zp)
                for t in range(NT):
                    j = t % 4
                    nc.tensor.matmul(
                        zp[32 * j : 32 * j + 2, :], wo_sb[:, t * 2 : (t + 1) * 2],
                        hout[:, t * BC + h * NB : t * BC + (h + 1) * NB],
                        start=(t < 4), stop=(t >= NT - 4),
                        tile_position=(0, 32 * j),
                    )
            for h in range(2):
                zs = zsbpool.tile([P, NB], F32, name=f"zs{h}")
                if h == 0:
                    nc.scalar.copy(zs[:], zps[h][:])
                else:
                    nc.vector.tensor_copy(zs[:], zps[h][:])
                nc.sync.dma_start(z_d[h], zs[:])

    nc.compile()
    return nc


def _build_strass():
    """One-level Strassen over the 4096x4096 layer matmuls (all 5 layers).

    W split 2x2 (2048 blocks), h split k-half x batch-half (quadrants, N
    stays 512 = one PSUM bank).  7 products per layer instead of 8: 1792
    MMs/layer vs 2048 -> ~55us/layer PE savings (~277us total).  The 7
    weight operands (entries {-2..2}, exact fp16) are host-precomputed; the
    5 activation combos are DVE adds (fp16) hidden under PE work, two of
    them stored in-place over the dead B12/B21 quadrants of the input tile.
    M order (M2,M5,M3,M4,M7,M6,M1) matches combo readiness (cb3,cb4,cb7,
    cb6,cb1).  Per ko: 7 PSUM M-tiles -> 8 DVE adds + 4 ACT relu evictions
    reassemble C11/C12/C21/C22 into the output quadrants.
    Activation tiles are quadrant-major: col = q*8192 + kt*512 + c with
    q = (k-half<<1)|batch-half; kt = k-tile index within the half (0..15).
    """
    nc = bacc.Bacc("TRN2", target_bir_lowering=False, debug=False)
    NQ = NT // 2      # 16 k/dout tiles per half
    QW = NQ * NB      # 8192 cols per quadrant
    xt_d = nc.dram_tensor("xt", [P, NT * BC], FP16, kind="ExternalInput").ap()
    wt_d = nc.dram_tensor("wt", [L, 7, NQ, P, NQ * P], FP16, kind="ExternalInput").ap()
    bias_d = nc.dram_tensor("biasd", [P, L * NT], F32, kind="ExternalInput").ap()
    wo_d = nc.dram_tensor("wo", [P, NT * 2], FP16, kind="ExternalInput").ap()
    z_d = nc.dram_tensor("z", [2, P, NB], F32, kind="ExternalOutput").ap()
    sc = float(2.0 ** (-FP16_SHIFT))
    # rhs operand per M exec slot: quadrant index of ain (in-place combos
    # land in q1/q2) or a dedicated combo tile (None here, filled below)
    #   e0=M2:q0(B11) e1=M5:q3(B22) e2=M3:cb3 e3=M4:cb4 e4=M7:cb7
    #   e5=M6:q2(<-cb6) e6=M1:q1(<-cb1)

    with tile.TileContext(nc) as tc:
        with (
            tc.tile_pool(name="acts", bufs=1) as acts,
            tc.tile_pool(name="combos", bufs=1) as cbpool,
            tc.tile_pool(name="wpool", bufs=4) as wpool,
            tc.tile_pool(name="const", bufs=1) as cpool,
            tc.tile_pool(name="warm", bufs=1) as warmpool,
            tc.tile_pool(name="ctmp", bufs=6) as ctpool,
            tc.tile_pool(name="mps", bufs=8, space="PSUM") as mpool,
        ):
            A = acts.tile([P, NT * BC], FP16, name="A")
            Bt = acts.tile([P, NT * BC], FP16, name="Bt")
            cb3 = cbpool.tile([P, QW], FP16, name="cb3")
            cb4 = cbpool.tile([P, QW], FP16, name="cb4")
            cb7 = cbpool.tile([P, QW], FP16, name="cb7")
            bias_sb = cpool.tile([P, L * NT], F32, name="bias_sb")
            wo_sb = cpool.tile([P, NT * 2], FP16, name="wo_sb")

            nc.scalar.dma_start(bias_sb[:], bias_d[:])
            nc.scalar.dma_start(wo_sb[:], wo_d[:])

            # startup: x quadrants (1MB pieces, 8KB lines) interleaved with
            # the first five layer-0 G-slabs on the sync ring, in M order.
            pre_keys = [(0, 0), (1, 0), (2, 0), (3, 0)]
            slab_pre = {}
            pre_it = iter(pre_keys)

            def preslab():
                k = next(pre_it, None)
                if k is not None:
                    s = wpool.tile([P, NQ * P], FP16, name="wslab")
                    nc.sync.dma_start(s[:], wt_d[0, k[0], k[1]])
                    slab_pre[k] = s

            def xpiece(q, half):
                c0 = q * QW + half * (QW // 2)
                nc.sync.dma_start(A[:, c0 : c0 + QW // 2], xt_d[:, c0 : c0 + QW // 2])

            xpiece(0, 0); preslab()          # B11 p0, G(M2,k0)
            xpiece(0, 1); preslab()          # B11 p1, G(M5,k0)
            xpiece(3, 0); xpiece(3, 1)       # B22
            preslab()                        # G(M3,k0)
            xpiece(1, 0); xpiece(1, 1)       # B12
            preslab()                        # G(M4,k0)
            xpiece(2, 0); xpiece(2, 1)       # B21
            preslab()                        # G(M2,k1)

            wt_warm = warmpool.tile([P, P + NB], FP16, name="wt_warm")
            nc.gpsimd.memset(wt_warm[:], 0.0)
            wp = mpool.tile([P, NB], F32, name="mt")
            for i in range(STRASS_WARMUP):
                nc.tensor.matmul(wp[:], wt_warm[:, 0:P], wt_warm[:, P:],
                                 start=(i == 0), stop=(i == STRASS_WARMUP - 1))
            # zp tiles pinned early (eager-drain keeps M pressure low) so
            # the output bursts never wait on the last ko's eviction chain;
            # they double as the filler target (unused until the end).
            zps = [mpool.tile([P, NB], F32, name="mt") for _ in range(2)]

            def filler(n):
                for _ in range(n):
                    nc.tensor.matmul(zps[0][:], wt_warm[:, 0:P], A[:, 0:NB],
                                     start=True, stop=True)

            def quad(t, q):
                return t[:, q * QW : (q + 1) * QW]

            for l in range(L):
                ain = A if l % 2 == 0 else Bt
                aout = Bt if l % 2 == 0 else A
                # combos (chunked x4 for finer deps); order matters: q2 is
                # read by cb4/cb7 before cb6 overwrites it, q1 by cb3/cb6
                # before cb1 overwrites it.
                CH4 = QW // 4
                for c in range(4):
                    s_ = slice(c * CH4, (c + 1) * CH4)
                    nc.vector.tensor_sub(cb3[:, s_], quad(ain, 1)[:, s_], quad(ain, 3)[:, s_])
                for c in range(4):
                    s_ = slice(c * CH4, (c + 1) * CH4)
                    nc.vector.tensor_sub(cb4[:, s_], quad(ain, 2)[:, s_], quad(ain, 0)[:, s_])
                for c in range(4):
                    s_ = slice(c * CH4, (c + 1) * CH4)
                    nc.vector.tensor_add(cb7[:, s_], quad(ain, 2)[:, s_], quad(ain, 3)[:, s_])
                for c in range(4):
                    s_ = slice(c * CH4, (c + 1) * CH4)
                    nc.vector.tensor_add(quad(ain, 2)[:, s_], quad(ain, 0)[:, s_], quad(ain, 1)[:, s_])
                for c in range(4):
                    s_ = slice(c * CH4, (c + 1) * CH4)
                    nc.vector.tensor_add(quad(ain, 1)[:, s_], quad(ain, 0)[:, s_], quad(ain, 3)[:, s_])
                rhs_ops = [quad(ain, 0), quad(ain, 3), cb3[:], cb4[:],
                           cb7[:], quad(ain, 2), quad(ain, 1)]
                for ko in range(NQ):
                    # Eager-drain: each M's PSUM is consumed into SBUF
                    # chains right after its 16 MMs (DVE tensor_tensor
                    # allows at most one PSUM operand), so only ~1 M bank
                    # is live at a time and the pinned zp tiles fit.
                    bc1 = bias_sb[:, l * NT + ko : l * NT + ko + 1]
                    bc2 = bias_sb[:, l * NT + NQ + ko : l * NT + NQ + ko + 1]

                    def ct():
                        return ctpool.tile([P, NB], F32, name="ct")

                    def act(q, src, bias):
                        nc.scalar.activation(
                            aout[:, q * QW + ko * NB : q * QW + (ko + 1) * NB],
                            src[:], mybir.ActivationFunctionType.Relu,
                            bias=bias, scale=sc)

                    st = {}
                    for e in range(7):
                        if l == 0:
                            # bridge the PE over DMA/combo waits ahead of
                            # this M-block (q1/q2/slabs still streaming in)
                            filler(STRASS_FILLERS.get((ko, e), 0))
                        if l == 0 and (e, ko) in slab_pre:
                            slab = slab_pre[(e, ko)]
                        else:
                            slab = wpool.tile([P, NQ * P], FP16, name="wslab")
                            nc.sync.dma_start(slab[:], wt_d[l, e, ko])
                        mt = mpool.tile([P, NB], F32, name="mt")
                        for j in range(NQ):
                            nc.tensor.matmul(
                                mt[:], slab[:, j * P : (j + 1) * P],
                                rhs_ops[e][:, j * NB : (j + 1) * NB],
                                start=(j == 0), stop=(j == NQ - 1),
                            )
                        if e == 0:      # m2
                            st["p1"] = ct()
                            nc.vector.tensor_copy(st["p1"][:], mt[:])
                        elif e == 1:    # m5
                            st["p3"] = ct()
                            nc.vector.tensor_copy(st["p3"][:], mt[:])
                        elif e == 2:    # m3
                            q12 = ct()
                            nc.vector.tensor_add(q12[:], st["p3"][:], mt[:])
                            act(1, q12, bc1)               # C12 = M3+M5
                            st["c22"] = ct()
                            nc.vector.tensor_sub(st["c22"][:], mt[:], st["p1"][:])
                        elif e == 3:    # m4
                            q21 = ct()
                            nc.vector.tensor_add(q21[:], st["p1"][:], mt[:])
                            act(2, q21, bc2)               # C21 = M2+M4
                            st["c11"] = ct()
                            nc.vector.tensor_sub(st["c11"][:], mt[:], st["p3"][:])
                        elif e == 4:    # m7
                            n11 = ct()
                            nc.vector.tensor_add(n11[:], st["c11"][:], mt[:])
                            st["c11"] = n11
                        elif e == 5:    # m6
                            n22 = ct()
                            nc.vector.tensor_add(n22[:], st["c22"][:], mt[:])
                            st["c22"] = n22
                        else:           # e == 6: m1
                            q11 = ct()
                            nc.vector.tensor_add(q11[:], st["c11"][:], mt[:])
                            act(0, q11, bc1)               # C11 = M4-M5+M7+M1
                            q22 = ct()
                            nc.vector.tensor_add(q22[:], st["c22"][:], mt[:])
                            act(3, q22, bc2)               # C22 = M3-M2+M6+M1

            # output layer: col-group packed, quadrant-aware rhs.  Tiles
            # t=15/31 read ko15's C-evictions (the very last ACTs) - issue
            # them LAST so the other 56 burst MMs overlap the eviction
            # chain instead of stalling behind t=15 in the PE FIFO.
            # start/stop are per (h, col-group): first/last issued in group.
            hout = A if L % 2 == 0 else Bt
            t_early = [t for t in range(NT) if t % NQ != NQ - 1]
            burst_list = [(h, t) for h in range(2) for t in t_early]
            burst_list += [(0, 15), (1, 31), (1, 15), (0, 31)]
            seen = {}
            for h, t in burst_list:
                j = t % 4
                q = (0 if t < NQ else 2) + h
                c0 = q * QW + (t % NQ) * NB
                k = (h, j)
                seen[k] = seen.get(k, 0) + 1
                nc.tensor.matmul(
                    zps[h][32 * j : 32 * j + 2, :], wo_sb[:, t * 2 : (t + 1) * 2],
                    hout[:, c0 : c0 + NB],
                    start=(seen[k] == 1), stop=(seen[k] == 8),
                    tile_position=(0, 32 * j),
                )
            for h in range(2):
                zs = ctpool.tile([P, NB], F32, name="ct")
                if h == 0:
                    nc.scalar.copy(zs[:], zps[h][:])
                else:
                    nc.vector.tensor_copy(zs[:], zps[h][:])
                nc.sync.dma_start(z_d[h], zs[:])

    nc.compile()
    return nc


def _build_hybrid():
    """fp16 hi + fp8e4m3 lo (DoubleRow) with 2^-6/layer activation rescale.

    h = hi + lo/LOSC; hi pass: 32 fp16 matmuls; lo pass: 16 fp8 DoubleRow
    matmuls (2 k-tiles each) into a separate PSUM bank, combined at evict.
    """
    FP8 = mybir.dt.float8e4
    nc = bacc.Bacc("TRN2", target_bir_lowering=False, debug=False)
    xth_d = nc.dram_tensor("xth", [CH, P, NT * NB], FP16, kind="ExternalInput").ap()
    xtl_d = nc.dram_tensor("xtl", [CH, P, NT * NB], FP8, kind="ExternalInput").ap()
    wt16_d = nc.dram_tensor("wt16", [L, NT, P, NT * P], FP16, kind="ExternalInput").ap()
    wt8_d = nc.dram_tensor("wt8", [L, NT, P, NT * P], FP8, kind="ExternalInput").ap()
    bias_d = nc.dram_tensor("biasd", [P, L * NT], F32, kind="ExternalInput").ap()
    wo16_d = nc.dram_tensor("wo16", [P, NT * 2], FP16, kind="ExternalInput").ap()
    wo8_d = nc.dram_tensor("wo8", [P, NT * 2], FP8, kind="ExternalInput").ap()
    z_d = nc.dram_tensor("z", [2, BC], F32, kind="ExternalOutput").ap()
    sc = float(2.0 ** (-FP16_SHIFT))

    with tile.TileContext(nc) as tc:
        with (
            tc.tile_pool(name="acts", bufs=1) as acts,
            tc.tile_pool(name="w16pool", bufs=3) as w16pool,
            tc.tile_pool(name="w8pool", bufs=3) as w8pool,
            tc.tile_pool(name="tmp", bufs=3) as tpool,
            tc.tile_pool(name="const", bufs=1) as cpool,
            tc.tile_pool(name="psumh", bufs=3, space="PSUM") as pph,
            tc.tile_pool(name="psuml", bufs=3, space="PSUM") as ppl,
            tc.tile_pool(name="zpsum", bufs=1, space="PSUM") as zppool,
            tc.tile_pool(name="zsb", bufs=2) as zsbpool,
        ):
            A_hi = acts.tile([P, NT * NB], FP16, name="A_hi")
            A_lo = acts.tile([P, NT * NB], FP8, name="A_lo")
            B_hi = acts.tile([P, NT * NB], FP16, name="B_hi")
            B_lo = acts.tile([P, NT * NB], FP8, name="B_lo")
            bias_sb = cpool.tile([P, L * NT], F32, name="bias_sb")
            wo16_sb = cpool.tile([P, NT * 2], FP16, name="wo16_sb")
            wo8_sb = cpool.tile([P, NT * 2], FP8, name="wo8_sb")
            nc.sync.dma_start(bias_sb[:], bias_d[:])
            nc.sync.dma_start(wo16_sb[:], wo16_d[:])
            nc.sync.dma_start(wo8_sb[:], wo8_d[:])

            for c in range(CH):
                nc.sync.dma_start(A_hi[:], xth_d[c])
                nc.sync.dma_start(A_lo[:], xtl_d[c])
                for l in range(L):
                    ain_hi, ain_lo = (A_hi, A_lo) if l % 2 == 0 else (B_hi, B_lo)
                    aout_hi, aout_lo = (B_hi, B_lo) if l % 2 == 0 else (A_hi, A_lo)
                    for ko in range(NT):
                        slab16 = w16pool.tile([P, NT * P], FP16, name="w16slab")
                        nc.sync.dma_start(slab16[:], wt16_d[l, ko])
                        slab8 = w8pool.tile([P, NT * P], FP8, name="w8slab")
                        nc.sync.dma_start(slab8[:], wt8_d[l, ko])
                        # NOTE: batched ordering (all fp16, then all DR) measures
                        # faster than hi,hi,lo interleave (219.4 vs 224.2 ns/MM):
                        # alternating Normal/DoubleRow perf modes thrashes the
                        # PE weight path more than the DR LDWEIGHTS costs.
                        pt = pph.tile([P, NB], F32, name="pt")
                        for d in range(NT):
                            nc.tensor.matmul(
                                pt[:], slab16[:, d * P : (d + 1) * P],
                                ain_hi[:, d * NB : (d + 1) * NB],
                                start=(d == 0), stop=(d == NT - 1),
                            )
                        plo = ppl.tile([P, NB], F32, name="plo")
                        for m in range(NT // 2):
                            lhsT = slab8[:, 2 * m * P : (2 * m + 2) * P].rearrange(
                                "p (j c) -> p j c", j=2
                            )
                            rhs = ain_lo[
                                :, 2 * m * NB : (2 * m + 2) * NB
                            ].rearrange("p (j b) -> p j b", j=2)
                            nc.tensor.matmul(
                                plo[:], lhsT, rhs,
                                start=(m == 0), stop=(m == NT // 2 - 1),
                                perf_mode=mybir.MatmulPerfMode.DoubleRow,
                            )
                        # combine + relu + re-split (t4s is relu result x LOSC)
                        t1 = tpool.tile([P, NB], F32, name="t1")
                        nc.scalar.mul(t1[:], plo[:], 1.0 / LOSC)
                        t2 = tpool.tile([P, NB], F32, name="t2")
                        nc.vector.tensor_add(t2[:], t1[:], pt[:])
                        t4s = tpool.tile([P, NB], F32, name="t4s")
                        nc.scalar.activation(
                            t4s[:], t2[:], mybir.ActivationFunctionType.Relu,
                            bias=bias_sb[:, l * NT + ko : l * NT + ko + 1],
                            scale=sc * LOSC,
                        )
                        nc.vector.tensor_scalar_mul(
                            aout_hi[:, ko * NB : (ko + 1) * NB], t4s[:], 1.0 / LOSC
                        )
                        nc.vector.scalar_tensor_tensor(
                            aout_lo[:, ko * NB : (ko + 1) * NB],
                            aout_hi[:, ko * NB : (ko + 1) * NB], -LOSC, t4s[:],
                            op0=mybir.AluOpType.mult, op1=mybir.AluOpType.add,
                        )

                hout_hi, hout_lo = (A_hi, A_lo) if L % 2 == 0 else (B_hi, B_lo)
                zph = zppool.tile([2, NB], F32, name="zph")
                for t in range(NT):
                    nc.tensor.matmul(
                        zph[:], wo16_sb[:, t * 2 : (t + 1) * 2],
                        hout_hi[:, t * NB : (t + 1) * NB],
                        start=(t == 0), stop=(t == NT - 1),
                    )
                zpl = zppool.tile([2, NB], F32, name="zpl")
                for t in range(NT):
                    nc.tensor.matmul(
                        zpl[:], wo8_sb[:, t * 2 : (t + 1) * 2],
                        hout_lo[:, t * NB : (t + 1) * NB],
                        start=(t == 0), stop=(t == NT - 1),
                    )
                zs1 = zsbpool.tile([2, NB], F32, name="zs1")
                nc.scalar.mul(zs1[:], zpl[:], 1.0 / LOSC)
                zs = zsbpool.tile([2, NB], F32, name="zs")
                nc.vector.tensor_add(zs[:], zs1[:], zph[:])
                nc.sync.dma_start(z_d[:, c * NB : (c + 1) * NB], zs[:])

    nc.compile()
    return nc


def _build_program(mode):
    if mode not in _BUILD_CACHE:
        _BUILD_CACHE[mode] = {
            "split2": _build_split2, "fp16": _build_fp16, "hybrid": _build_hybrid,
            "fp16m": _build_fp16m, "fp16s": _build_fp16s, "strass": _build_strass,
        }[mode]()
    return _BUILD_CACHE[mode]


def _split_hilo(a32: np.ndarray):
    hi = a32.astype(BF)
    lo = (a32 - hi.astype(np.float32)).astype(BF)
    return hi, lo


def _weights_layout(w: np.ndarray) -> np.ndarray:
    """[dout, din] f32 -> [NT(ko), P(p), NT*P(t*128+c)]."""
    return w.reshape(NT, P, NT, P).transpose(0, 3, 2, 1).reshape(NT, P, NT * P)


def _prepare_inputs(mode, x, scores1_h, scores2_h, bias_h, scores1_o, scores2_o):
    """Host-side: masks, weight/bias/x layouts for the device program.

    Returns a list of per-core in_maps (without only the core-varying xt)."""
    F8 = ml_dtypes.float8_e4m3
    wdt = BF if mode == "split2" else F16
    if mode == "strass":
        NQ, Q = NT // 2, D // 2
        QW = NQ * NB

        def _wl2048(w):
            return w.reshape(NQ, P, NQ, P).transpose(0, 3, 2, 1).reshape(NQ, P, NQ * P)

        wt = np.empty((L, 7, NQ, P, NQ * P), F16)
        for l in range(L):
            W = _subnet_mask(scores1_h[l]) - _subnet_mask(scores2_h[l])
            A11, A12 = W[:Q, :Q], W[:Q, Q:]
            A21, A22 = W[Q:, :Q], W[Q:, Q:]
            # exec order M2,M5,M3,M4,M7,M6,M1 (combos {-2..2} exact in fp16)
            Gs = (A21 + A22, A11 + A12, A11, A22, A12 - A22, A21 - A11, A11 + A22)
            for i, G in enumerate(Gs):
                wt[l, i] = _wl2048(G).astype(F16)
        scales = 2.0 ** (-FP16_SHIFT * np.arange(1, L + 1, dtype=np.float32))
        b2 = 2.0 * bias_h.astype(np.float32) * scales[:, None]
        bias_sb = np.ascontiguousarray(
            b2.reshape(L, NT, P).transpose(2, 0, 1).reshape(P, L * NT)
        )
        wo = _subnet_mask(scores1_o) - _subnet_mask(scores2_o)
        wo_sb = np.ascontiguousarray(
            wo.reshape(2, NT, P).transpose(2, 1, 0).reshape(P, NT * 2)
        ).astype(F16)
        common = {"wt": wt, "biasd": bias_sb, "wo": wo_sb}
        in_maps = []
        for i in range(N_CORES):
            xT = x[i * BC : (i + 1) * BC].T.astype(np.float32)  # [D, BC]
            blocks = [xT[:Q, :NB], xT[:Q, NB:], xT[Q:, :NB], xT[Q:, NB:]]
            xr = np.concatenate(
                [b.reshape(NQ, P, NB).transpose(1, 0, 2).reshape(P, QW)
                 for b in blocks], axis=1,
            ).astype(F16)
            in_maps.append({**common, "xt": np.ascontiguousarray(xr)})
        return in_maps
    if mode == "fp16s":
        wt = np.empty((L, NT, P, NT * P), F16)
        for l in range(L):
            w = _subnet_mask(scores1_h[l]) - _subnet_mask(scores2_h[l])
            wt[l] = _weights_layout(w).astype(F16)
        scales = 2.0 ** (-FP16_SHIFT * np.arange(1, L + 1, dtype=np.float32))
        b2 = 2.0 * bias_h.astype(np.float32) * scales[:, None]
        bias_sb = np.ascontiguousarray(
            b2.reshape(L, NT, P).transpose(2, 0, 1).reshape(P, L * NT)
        )
        wo = _subnet_mask(scores1_o) - _subnet_mask(scores2_o)
        wo_sb = np.ascontiguousarray(
            wo.reshape(2, NT, P).transpose(2, 1, 0).reshape(P, NT * 2)
        ).astype(F16)
        common = {"wt": wt, "biasd": bias_sb, "wo": wo_sb}
        in_maps = []
        for i in range(N_CORES):
            xT = x[i * BC : (i + 1) * BC].T.astype(np.float32)  # [D, BC]
            # half-major layout: [2(half), P, NT*NB], col = d*NB + c
            xr = np.ascontiguousarray(
                xT.reshape(NT, P, 2, NB).transpose(2, 1, 0, 3).reshape(2, P, NT * NB)
            ).astype(F16)
            in_maps.append({**common, "xt": xr})
        return in_maps
    if mode == "fp16m":
        wt = np.empty((L, NT, P, NT * P), F16)
        for l in range(L):
            w = _subnet_mask(scores1_h[l]) - _subnet_mask(scores2_h[l])
            wt[l] = _weights_layout(w).astype(F16)
        scales = 2.0 ** (-FP16_SHIFT * np.arange(1, L + 1, dtype=np.float32))
        b2 = 2.0 * bias_h.astype(np.float32) * scales[:, None]
        bias_sb = np.ascontiguousarray(
            b2.reshape(L, NT, P).transpose(2, 0, 1).reshape(P, L * NT)
        )
        wo = _subnet_mask(scores1_o) - _subnet_mask(scores2_o)
        wo_sb = np.ascontiguousarray(
            wo.reshape(2, NT, P).transpose(2, 1, 0).reshape(P, NT * 2)
        ).astype(F16)
        sel_np = np.zeros((P, 2), F16)
        for j in range(4):
            sel_np[32 * j, 0] = 1.0
            sel_np[32 * j + 1, 1] = 1.0
        common = {"wt": wt, "biasd": bias_sb, "wo": wo_sb, "sel": sel_np}
        in_maps = []
        for i in range(N_CORES):
            xT = x[i * BC : (i + 1) * BC].T.astype(np.float32)  # [D, BC]
            xr = np.ascontiguousarray(
                xT.reshape(NT, P, BC).transpose(1, 0, 2).reshape(P, NT * BC)
            ).astype(F16)
            in_maps.append({**common, "xt": xr})
        return in_maps
    wt = np.empty((L, NT, P, NT * P), wdt)
    wt8 = np.empty((L, NT, P, NT * P), F8) if mode == "hybrid" else None
    for l in range(L):
        w = _subnet_mask(scores1_h[l]) - _subnet_mask(scores2_h[l])  # [dout, din]
        wl = _weights_layout(w)
        wt[l] = wl.astype(wdt)
        if wt8 is not None:
            wt8[l] = wl.astype(F8)

    b2 = 2.0 * bias_h.astype(np.float32)  # [L, D]
    if mode in ("fp16", "hybrid"):
        scales = (2.0 ** (-FP16_SHIFT * np.arange(1, L + 1, dtype=np.float32)))
        b2 = b2 * scales[:, None]
        if mode == "hybrid":
            b2 = b2 * LOSC
    bias_sb = np.ascontiguousarray(
        b2.reshape(L, NT, P).transpose(2, 0, 1).reshape(P, L * NT)
    )

    wo = _subnet_mask(scores1_o) - _subnet_mask(scores2_o)  # [2, D]
    wo_l = np.ascontiguousarray(
        wo.reshape(2, NT, P).transpose(2, 1, 0).reshape(P, NT * 2)
    )
    wo_sb = wo_l.astype(wdt)

    common = {}
    if mode == "hybrid":
        common = {"wt16": wt, "wt8": wt8, "biasd": bias_sb,
                  "wo16": wo_sb, "wo8": wo_l.astype(F8)}
    else:
        common = {"wt": wt, "biasd": bias_sb, "wo": wo_sb}

    in_maps = []
    for i in range(N_CORES):
        xT = x[i * BC : (i + 1) * BC].T.astype(np.float32)  # [D, BC]
        xr = xT.reshape(NT, P, CH, NB).transpose(2, 1, 0, 3).reshape(CH, P, NT * NB)
        xr = np.ascontiguousarray(xr)
        if mode == "split2":
            hi, lo = _split_hilo(xr)
            xt = {"xt": np.ascontiguousarray(np.stack([hi, lo], axis=1))}
        elif mode == "fp16":
            xt = {"xt": xr.astype(F16)}
        else:
            hi = xr.astype(F16)
            lo8 = ((xr - hi.astype(np.float32)) * LOSC).astype(F8)
            xt = {"xth": hi, "xtl": lo8}
        in_maps.append({**common, **xt})
    return in_maps


def kernel(x, scores1_h, scores2_h, bias_h, scores1_o, scores2_o, bias_o, w_last,
           _trace=False, _run_kwargs=None):
    x = np.asarray(x, np.float32)
    scores1_h = np.asarray(scores1_h, np.float32)
    scores2_h = np.asarray(scores2_h, np.float32)
    bias_h = np.asarray(bias_h, np.float32)
    scores1_o = np.asarray(scores1_o, np.float32)
    scores2_o = np.asarray(scores2_o, np.float32)
    bias_o = np.asarray(bias_o, np.float32)
    w_last = np.asarray(w_last, np.float32)

    in_maps = _prepare_inputs(
        MODE, x, scores1_h, scores2_h, bias_h, scores1_o, scores2_o
    )
    nc = _build_program(MODE)
    # Retry guard: very rare transient HW faults have been observed to produce
    # NaN output (z is tiny, so the check is free). Clean runs are bit-identical.
    _zidx = [0, 1, 32, 33, 64, 65, 96, 97]  # fp16s: used partial partitions
    for attempt in range(3):
        res = run_bass_kernel_spmd(
            nc, in_maps, core_ids=list(range(N_CORES)), trace=_trace,
            **(_run_kwargs or {}),
        )
        zs_all = np.stack([res.results[i]["z"] for i in range(N_CORES)])
        if MODE in ("fp16s", "strass"):
            zs_chk = zs_all[:, :, _zidx, :]  # other partitions are junk
        else:
            zs_chk = zs_all
        zbound = 1e11 if MODE == "split2" else 1e4  # z is 2^-30-scaled otherwise
        if np.isfinite(zs_chk).all() and np.abs(zs_chk).max() < zbound:
            break
        print(f"kernel: bad z detected (attempt {attempt}), retrying", file=sys.stderr)

    # host-side tail: relu(z.T + 2*b_o) @ w_last.T  (tiny: [8192, 2] -> [8192, 1])
    zscale = 1.0 if MODE == "split2" else float(2.0 ** (FP16_SHIFT * L))
    # (fp16/hybrid carry h5 scaled by 2^-30; z output is pre-bias, pre-relu)
    y = np.empty((B, 1), np.float32)
    for i in range(N_CORES):
        if MODE in ("fp16s", "strass"):
            zr = res.results[i]["z"]  # [2(half), P, NB] f32, 4 partials/output
            z2 = zr[:, _zidx, :].reshape(2, 4, 2, NB).sum(axis=1)  # [half, o, c]
            z = np.concatenate([z2[0].T, z2[1].T], axis=0)  # [BC, 2]
        else:
            z = res.results[i]["z"].astype(np.float32).T  # [BC, 2]
        h = np.maximum(z * zscale + 2.0 * bias_o[None, :], 0.0).astype(np.float32)
        y[i * BC : (i + 1) * BC] = h @ w_last.T
    if _trace:
        kernel.last_results = res
    return y



# revision 34
# speedup vs baseline: 1.0018x; 1.0018x over previous
"""Trainium2 Bass kernel for nn_DoubleNet (topk_masking).

Computation (see reference):
  5 hidden layers: h = relu(h @ (m1_l - m2_l).T + 2*b_l)   [8192, 4096]
  output layer:    h = relu(h @ (m1_o - m2_o).T + 2*b_o)   [8192, 2]
  final:           y = h @ w_last.T                        [8192, 1]
where m1/m2 are top-50% masks of |scores| (exact argsort tie semantics).

Strategy:
  - Masks are x-independent: computed exactly on host with an O(n)
    partition + stable tie-fix that matches jnp.argsort(stable) bit-exactly.
  - Data-parallel: batch 8192 split as 1024 rows per NeuronCore (8 cores).
  - Weights w = m1 - m2 in {-1, 0, +1} are exact in fp16.
  - Per core, all 1024 activation cols stay resident in SBUF through all
    layers; weight slabs stream from HBM (168MB/core), triple-buffered,
    each slab reused for both 512-col batch halves.

MODE:
  "strass" (default): one-level Strassen over every 4096x4096 layer (W
          split 2x2, h split k-half x batch-half): 7 products instead of
          8 -> 1792 MMs/layer vs 2048, beating the classical fp16 PE
          "roofline" by ~12%. Weight combos (entries {-2..2}) are exact
          in fp16 and host-precomputed; activation combos are DVE adds
          hidden under PE work; eager-drain C-recombination keeps PSUM
          pressure at ~1 bank. ~3.6e-3 rel err (gate is 2e-2), ~1.97ms.
  "fp16s": classical fp16 single pass at the PE streaming floor
          (~2.24ms): 1 rhs column/cycle at N=512 regardless of
          dtype/perf-mode (measured: fp16=bf16=fp8DR ~216ns/MM warm);
          pure-fp8 DoubleRow would be 2x but fails the 2e-2 gate (6e-2),
          and any hi+lo fp8 correction costs exactly fp16. ~1.8e-3 err.
  "fp16m": fp16s predecessor (merged batch, no edge tuning).
  "split2": hi/lo bf16, 2 matmul passes, ~2.5e-5 rel err (2x slower).
  "fp16": like fp16m but 2 batch chunks of 512 (2x weight DMA).
  "hybrid": fp16 + fp8 DoubleRow lo pass, ~4.6e-5 rel err (1.5x slower).
"""

import sys

for _p in ("/opt/trn_rl_repo", "/root/.axon_site/_ro/trn_rl_repo"):
    if _p not in sys.path:
        sys.path.insert(0, _p)

import numpy as np
import ml_dtypes

import concourse.mybir as mybir
import concourse.tile as tile
from concourse import bacc
from concourse.bass_utils import run_bass_kernel_spmd

BF = ml_dtypes.bfloat16
F16 = np.float16
BF16 = mybir.dt.bfloat16
FP16 = mybir.dt.float16
F32 = mybir.dt.float32

P = 128          # partitions
N_CORES = 8
B = 8192         # total batch
D = 4096         # width
L = 5            # hidden layers
KEEP = 0.5
NT = D // P      # 32 d/ko tiles
BC = B // N_CORES  # 1024 batch rows per core
NB = 512         # matmul free dim (one PSUM bank of fp32)
CH = BC // NB    # 2 chunks per core

MODE = "strass"  # "split2" | "fp16" | "hybrid" | "fp16m" | "fp16s" | "strass"
FP16_SHIFT = 6   # fp16/hybrid: h_l is carried scaled by 2^(-FP16_SHIFT*l)
LOSC = 8192.0
WARMUP_MMS = 16   # fp16m: PE warmup matmuls during the x load
WARMUP_MMS_S = 14  # fp16s: warmup sized to end ~ when piece0+slab0 land
KO0_FILLERS = 8    # fp16s: discard MMs inside (l0,h0,ko0), paced by x pieces
POST_KO0_FILLERS = 8  # fp16s: discard MMs between ko0 and ko1 (slab1 DMA gap)
STRASS_WARMUP = 16    # strass: PE warmup MMs during the x/slab load
# strass: filler MMs before (ko, exec-slot) of layer 0, bridging DMA waits
STRASS_FILLERS = {(0, 1): 4, (0, 2): 12, (0, 3): 12, (0, 4): 6, (0, 5): 6,
                  (0, 6): 6, (1, 0): 4}
# LOSC: hybrid mode: lo residual is stored as fp8 scaled by 2^13

_BUILD_CACHE = {}


def _subnet_mask(scores: np.ndarray) -> np.ndarray:
    """Exact replica of reference.get_subnet(|scores|) forward value.

    Zero the j smallest |scores| (ties at the threshold broken by flat
    index order, matching stable argsort), one elsewhere.
    """
    flat = np.abs(scores.astype(np.float32, copy=False)).ravel()
    n = flat.size
    j = int((1.0 - KEEP) * n)
    if j == 0:
        return np.ones(scores.shape, np.float32)
    thr = np.partition(flat, j - 1)[j - 1]
    mask = (flat > thr).astype(np.float32)
    c_lt = int((flat < thr).sum())
    idx_eq = np.flatnonzero(flat == thr)
    n_zero_eq = j - c_lt
    assert 0 <= n_zero_eq <= idx_eq.size
    mask[idx_eq[n_zero_eq:]] = 1.0
    return mask.reshape(scores.shape)


def _build_split2():
    nc = bacc.Bacc("TRN2", target_bir_lowering=False, debug=False)
    xt_d = nc.dram_tensor("xt", [CH, 2, P, NT * NB], BF16, kind="ExternalInput").ap()
    wt_d = nc.dram_tensor("wt", [L, NT, P, NT * P], BF16, kind="ExternalInput").ap()
    bias_d = nc.dram_tensor("biasd", [P, L * NT], F32, kind="ExternalInput").ap()
    wo_d = nc.dram_tensor("wo", [P, NT * 2], BF16, kind="ExternalInput").ap()
    z_d = nc.dram_tensor("z", [2, BC], F32, kind="ExternalOutput").ap()

    with tile.TileContext(nc) as tc:
        with (
            tc.tile_pool(name="acts", bufs=1) as acts,
            tc.tile_pool(name="wpool", bufs=3) as wpool,
            tc.tile_pool(name="tmp", bufs=3) as tpool,
            tc.tile_pool(name="const", bufs=1) as cpool,
            tc.tile_pool(name="psum", bufs=2, space="PSUM") as ppool,
            tc.tile_pool(name="wps", bufs=1, space="PSUM") as wppool,
            tc.tile_pool(name="zpsum", bufs=2, space="PSUM") as zppool,
            tc.tile_pool(name="zsb", bufs=2) as zsbpool,
        ):
            A_hi = acts.tile([P, NT * NB], BF16, name="A_hi")
            A_lo = acts.tile([P, NT * NB], BF16, name="A_lo")
            B_hi = acts.tile([P, NT * NB], BF16, name="B_hi")
            B_lo = acts.tile([P, NT * NB], BF16, name="B_lo")
            bias_sb = cpool.tile([P, L * NT], F32, name="bias_sb")
            wo_sb = cpool.tile([P, NT * 2], BF16, name="wo_sb")
            nc.sync.dma_start(bias_sb[:], bias_d[:])
            nc.sync.dma_start(wo_sb[:], wo_d[:])

            for c in range(CH):
                nc.sync.dma_start(A_hi[:], xt_d[c, 0])
                nc.sync.dma_start(A_lo[:], xt_d[c, 1])
                for l in range(L):
                    ain_hi, ain_lo = (A_hi, A_lo) if l % 2 == 0 else (B_hi, B_lo)
                    aout_hi, aout_lo = (B_hi, B_lo) if l % 2 == 0 else (A_hi, A_lo)
                    for ko in range(NT):
                        slab = wpool.tile([P, NT * P], BF16, name="wslab")
                        nc.sync.dma_start(slab[:], wt_d[l, ko])
                        pt = ppool.tile([P, NB], F32, name="pt")
                        for d in range(NT):
                            lhsT = slab[:, d * P : (d + 1) * P]
                            nc.tensor.matmul(
                                pt[:], lhsT, ain_hi[:, d * NB : (d + 1) * NB],
                                start=(d == 0), stop=False,
                            )
                            nc.tensor.matmul(
                                pt[:], lhsT, ain_lo[:, d * NB : (d + 1) * NB],
                                start=False, stop=(d == NT - 1),
                            )
                        tmp = tpool.tile([P, NB], F32, name="tmp")
                        nc.scalar.activation(
                            tmp[:], pt[:], mybir.ActivationFunctionType.Relu,
                            bias=bias_sb[:, l * NT + ko : l * NT + ko + 1], scale=1.0,
                        )
                        nc.vector.tensor_copy(
                            aout_hi[:, ko * NB : (ko + 1) * NB], tmp[:]
                        )
                        nc.vector.tensor_sub(
                            aout_lo[:, ko * NB : (ko + 1) * NB],
                            tmp[:],
                            aout_hi[:, ko * NB : (ko + 1) * NB],
                        )

                # output layer: z[2, NB] = w_o @ h5 (pre-bias, pre-relu)
                hout_hi, hout_lo = (A_hi, A_lo) if L % 2 == 0 else (B_hi, B_lo)
                zp = zppool.tile([2, NB], F32, name="zp")
                for t in range(NT):
                    lhsT = wo_sb[:, t * 2 : (t + 1) * 2]
                    nc.tensor.matmul(
                        zp[:], lhsT, hout_hi[:, t * NB : (t + 1) * NB],
                        start=(t == 0), stop=False,
                    )
                    nc.tensor.matmul(
                        zp[:], lhsT, hout_lo[:, t * NB : (t + 1) * NB],
                        start=False, stop=(t == NT - 1),
                    )
                zs = zsbpool.tile([2, NB], F32, name="zs")
                nc.vector.tensor_copy(zs[:], zp[:])
                nc.sync.dma_start(z_d[:, c * NB : (c + 1) * NB], zs[:])

    nc.compile()
    return nc


def _build_fp16():
    nc = bacc.Bacc("TRN2", target_bir_lowering=False, debug=False)
    xt_d = nc.dram_tensor("xt", [CH, P, NT * NB], FP16, kind="ExternalInput").ap()
    wt_d = nc.dram_tensor("wt", [L, NT, P, NT * P], FP16, kind="ExternalInput").ap()
    bias_d = nc.dram_tensor("biasd", [P, L * NT], F32, kind="ExternalInput").ap()
    wo_d = nc.dram_tensor("wo", [P, NT * 2], FP16, kind="ExternalInput").ap()
    z_d = nc.dram_tensor("z", [2, BC], F32, kind="ExternalOutput").ap()
    sc = float(2.0 ** (-FP16_SHIFT))

    with tile.TileContext(nc) as tc:
        with (
            tc.tile_pool(name="acts", bufs=1) as acts,
            tc.tile_pool(name="wpool", bufs=3) as wpool,
            tc.tile_pool(name="const", bufs=1) as cpool,
            tc.tile_pool(name="psum", bufs=2, space="PSUM") as ppool,
            tc.tile_pool(name="wps", bufs=1, space="PSUM") as wppool,
            tc.tile_pool(name="zpsum", bufs=2, space="PSUM") as zppool,
            tc.tile_pool(name="zsb", bufs=2) as zsbpool,
        ):
            A = acts.tile([P, NT * NB], FP16, name="A")
            Bt = acts.tile([P, NT * NB], FP16, name="Bt")
            bias_sb = cpool.tile([P, L * NT], F32, name="bias_sb")
            wo_sb = cpool.tile([P, NT * 2], FP16, name="wo_sb")
            nc.sync.dma_start(bias_sb[:], bias_d[:])
            nc.sync.dma_start(wo_sb[:], wo_d[:])

            for c in range(CH):
                nc.sync.dma_start(A[:], xt_d[c])
                for l in range(L):
                    ain = A if l % 2 == 0 else Bt
                    aout = Bt if l % 2 == 0 else A
                    for ko in range(NT):
                        slab = wpool.tile([P, NT * P], FP16, name="wslab")
                        nc.sync.dma_start(slab[:], wt_d[l, ko])
                        pt = ppool.tile([P, NB], F32, name="pt")
                        for d in range(NT):
                            nc.tensor.matmul(
                                pt[:], slab[:, d * P : (d + 1) * P],
                                ain[:, d * NB : (d + 1) * NB],
                                start=(d == 0), stop=(d == NT - 1),
                            )
                        # g_{l+1} = relu(2^-S * psum + 2*b*2^(-S(l+1))), fp16 out
                        nc.scalar.activation(
                            aout[:, ko * NB : (ko + 1) * NB], pt[:],
                            mybir.ActivationFunctionType.Relu,
                            bias=bias_sb[:, l * NT + ko : l * NT + ko + 1], scale=sc,
                        )

                hout = A if L % 2 == 0 else Bt
                zp = zppool.tile([2, NB], F32, name="zp")
                for t in range(NT):
                    nc.tensor.matmul(
                        zp[:], wo_sb[:, t * 2 : (t + 1) * 2],
                        hout[:, t * NB : (t + 1) * NB],
                        start=(t == 0), stop=(t == NT - 1),
                    )
                zs = zsbpool.tile([2, NB], F32, name="zs")
                nc.vector.tensor_copy(zs[:], zp[:])
                nc.sync.dma_start(z_d[:, c * NB : (c + 1) * NB], zs[:])

    nc.compile()
    return nc


def _build_fp16m():
    """fp16 single-pass, merged batch (1024 cols/core in one sweep).

    vs _build_fp16: weights are loaded once per (l, ko) slab and used for
    both 512-col batch halves (halves HBM weight traffic to 168MB/core and
    removes the inter-chunk PE gap); x is DMA'd in 4 pieces so layer 0 can
    start before the full 8MB lands.
    """
    nc = bacc.Bacc("TRN2", target_bir_lowering=False, debug=False)
    xt_d = nc.dram_tensor("xt", [P, NT * BC], FP16, kind="ExternalInput").ap()
    wt_d = nc.dram_tensor("wt", [L, NT, P, NT * P], FP16, kind="ExternalInput").ap()
    bias_d = nc.dram_tensor("biasd", [P, L * NT], F32, kind="ExternalInput").ap()
    wo_d = nc.dram_tensor("wo", [P, NT * 2], FP16, kind="ExternalInput").ap()
    sel_d = nc.dram_tensor("sel", [P, 2], FP16, kind="ExternalInput").ap()
    z_d = nc.dram_tensor("z", [2, BC], F32, kind="ExternalOutput").ap()
    sc = float(2.0 ** (-FP16_SHIFT))
    XP = 8  # x DMA pieces

    with tile.TileContext(nc) as tc:
        with (
            tc.tile_pool(name="acts", bufs=1) as acts,
            tc.tile_pool(name="wpool", bufs=3) as wpool,
            tc.tile_pool(name="const", bufs=1) as cpool,
            tc.tile_pool(name="warm", bufs=1) as warmpool,
            tc.tile_pool(name="psum", bufs=2, space="PSUM") as ppool,
            tc.tile_pool(name="wps", bufs=1, space="PSUM") as wppool,
            tc.tile_pool(name="zpsum", bufs=2, space="PSUM") as zppool,
            tc.tile_pool(name="zsb", bufs=2) as zsbpool,
        ):
            A = acts.tile([P, NT * BC], FP16, name="A")
            Bt = acts.tile([P, NT * BC], FP16, name="Bt")
            bias_sb = cpool.tile([P, L * NT], F32, name="bias_sb")
            wo_sb = cpool.tile([P, NT * 2], FP16, name="wo_sb")
            xpc = NT * BC // XP
            # DMA issue order matters: the rings drain in order, so the
            # first weight slab + consts must not queue behind all 8MB of
            # x (that cost a 15us PE stall). piece0 -> slab0 -> consts ->
            # remaining x pieces.
            nc.sync.dma_start(A[:, 0:xpc], xt_d[:, 0:xpc])
            slab0 = wpool.tile([P, NT * P], FP16, name="wslab")
            nc.sync.dma_start(slab0[:], wt_d[0, 0])
            nc.sync.dma_start(bias_sb[:], bias_d[:])
            nc.sync.dma_start(wo_sb[:], wo_d[:])
            for pc in range(1, XP):
                nc.sync.dma_start(
                    A[:, pc * xpc : (pc + 1) * xpc],
                    xt_d[:, pc * xpc : (pc + 1) * xpc],
                )

            # PE warmup while x/weights stream in: keeps the HAM activity
            # window busy (and the PE instruction queue deep) so the first
            # real matmuls run at 2.4GHz with no sync micro-gaps. Sized to
            # roughly bridge the ~22us x-load.
            wt_warm = warmpool.tile([P, P + NB], FP16, name="wt_warm")
            nc.vector.memset(wt_warm[:], 0.0)
            wp = wppool.tile([P, NB], F32, name="wp")
            for i in range(WARMUP_MMS):
                nc.tensor.matmul(wp[:], wt_warm[:, 0:P], wt_warm[:, P:],
                                 start=(i == 0), stop=(i == WARMUP_MMS - 1))

            for l in range(L):
                ain = A if l % 2 == 0 else Bt
                aout = Bt if l % 2 == 0 else A
                for ko in range(NT):
                    if l == 0 and ko == 0:
                        slab = slab0
                    else:
                        slab = wpool.tile([P, NT * P], FP16, name="wslab")
                        nc.sync.dma_start(slab[:], wt_d[l, ko])
                    # both batch halves interleaved per k-tile (one 2-bank
                    # psum tile; each MM's out slice stays within a bank):
                    # layer 0 then paces both halves with the arriving x
                    # pieces instead of re-running half after the x window
                    pt = ppool.tile([P, 2 * NB], F32, name="pt")
                    for d in range(NT):
                        for h in range(2):
                            nc.tensor.matmul(
                                pt[:, h * NB : (h + 1) * NB],
                                slab[:, d * P : (d + 1) * P],
                                ain[:, d * BC + h * NB : d * BC + (h + 1) * NB],
                                start=(d == 0), stop=(d == NT - 1),
                            )
                    for h in range(2):
                        nc.scalar.activation(
                            aout[:, ko * BC + h * NB : ko * BC + (h + 1) * NB],
                            pt[:, h * NB : (h + 1) * NB],
                            mybir.ActivationFunctionType.Relu,
                            bias=bias_sb[:, l * NT + ko : l * NT + ko + 1], scale=sc,
                        )

            # Output layer, col-group packed: the [128k, 2out] matmuls use
            # only 2 of 128 PE columns, so run 4 k-tiles concurrently in
            # col groups {0,32,64,96} (4 partial z pairs), then reduce the
            # 4 partials across partitions with a tiny selection matmul.
            hout = A if L % 2 == 0 else Bt
            zevs = []
            for h in range(2):
                zev = cpool.tile([P, NB], FP16, name=f"zev{h}")
                nc.vector.memset(zev[:], 0.0)
                zevs.append(zev)
            sel = cpool.tile([P, 2], FP16, name="sel")
            nc.sync.dma_start(sel[:], sel_d[:])
            # both packed groups back-to-back on the PE; the per-half DVE
            # evictions overlap the other half's matmuls, combines at the end
            for h in range(2):
                zp4 = zppool.tile([P, NB], F32, name="zp4")
                for t in range(NT):
                    j = t % 4
                    nc.tensor.matmul(
                        zp4[32 * j : 32 * j + 2, :], wo_sb[:, t * 2 : (t + 1) * 2],
                        hout[:, t * BC + h * NB : t * BC + (h + 1) * NB],
                        start=(t < 4), stop=(t >= NT - 4),
                        tile_position=(0, 32 * j),
                    )
                for j in range(4):
                    nc.vector.tensor_copy(
                        zevs[h][32 * j : 32 * j + 2, :], zp4[32 * j : 32 * j + 2, :]
                    )
            for h in range(2):
                zpf = wppool.tile([2, NB], F32, name="zpf")
                nc.tensor.matmul(zpf[:], sel[:], zevs[h][:], start=True, stop=True)
                zs = zsbpool.tile([2, NB], F32, name="zs")
                nc.vector.tensor_copy(zs[:], zpf[:])
                nc.sync.dma_start(z_d[:, h * NB : (h + 1) * NB], zs[:])

    nc.compile()
    return nc


def _build_fp16s():
    """fp16 single-pass like fp16m, tuned at the edges (trace-driven).

    vs _build_fp16m:
      - Startup is DMA-BW-bound (~420GB/s aggregate): layer 0 runs as two
        512-col half-sweeps (slab reloaded per half), and the first THREE
        ko groups of the h0 sweep are interleaved d-wise with staggered
        offsets - they all read the same arriving x pieces, so the PE gets
        3x the work per landed x byte and is never starved while x-half0
        (4MB) streams in.  x is stored half-major [2, P, NT*NB]; slab0 is
        chunked so the very first MM only needs ~0.26MB.  bias/wo go on
        the scalar engine's DMA queue (idle at startup).  Discardable
        filler MMs cover the residual DMA-behind instants.
      - Warmup lhsT memset on gpsimd (free ~1us earlier than DVE).
      - Output layer: per-half col-group-packed partial bursts are
        interleaved into the last hidden layer's ko loop (only the last
        burst + eviction remain after the final matmul); each [128,512]
        f32 PSUM tile is evicted whole (scalar h0 / vector h1, runs
        concurrently) and DMA'd raw - host sums partitions {32j,32j+1}.
    """
    nc = bacc.Bacc("TRN2", target_bir_lowering=False, debug=False)
    xt_d = nc.dram_tensor("xt", [2, P, NT * NB], FP16, kind="ExternalInput").ap()
    wt_d = nc.dram_tensor("wt", [L, NT, P, NT * P], FP16, kind="ExternalInput").ap()
    bias_d = nc.dram_tensor("biasd", [P, L * NT], F32, kind="ExternalInput").ap()
    wo_d = nc.dram_tensor("wo", [P, NT * 2], FP16, kind="ExternalInput").ap()
    z_d = nc.dram_tensor("z", [2, P, NB], F32, kind="ExternalOutput").ap()
    sc = float(2.0 ** (-FP16_SHIFT))
    XP = 8           # x DMA pieces per half (0.5MB each = 4 d-tiles)
    HNB = NT * NB    # columns per half in the layer-0 x layout

    with tile.TileContext(nc) as tc:
        with (
            tc.tile_pool(name="acts", bufs=1) as acts,
            tc.tile_pool(name="wpool", bufs=3) as wpool,
            tc.tile_pool(name="const", bufs=1) as cpool,
            tc.tile_pool(name="warm", bufs=1) as warmpool,
            tc.tile_pool(name="psum", bufs=2, space="PSUM") as ppool,
            tc.tile_pool(name="wps", bufs=1, space="PSUM") as wppool,
            tc.tile_pool(name="zpsum", bufs=1, space="PSUM") as zpool,
            tc.tile_pool(name="zsb", bufs=1) as zsbpool,
        ):
            A = acts.tile([P, NT * BC], FP16, name="A")
            Bt = acts.tile([P, NT * BC], FP16, name="Bt")
            bias_sb = cpool.tile([P, L * NT], F32, name="bias_sb")
            wo_sb = cpool.tile([P, NT * 2], FP16, name="wo_sb")
            xpc = HNB // XP
            # consts go on the scalar engine's hw DMA queue (idle at start);
            # sync-queue order: x-half0 pieces first (ko0 paces behind them),
            # slab0 right after piece0, then slabs 1-2; x-half1 pieces
            # interleave into the h0 ko-loop below.
            nc.scalar.dma_start(bias_sb[:], bias_d[:])
            nc.scalar.dma_start(wo_sb[:], wo_d[:])
            nc.sync.dma_start(A[:, 0:xpc], xt_d[0, :, 0:xpc])
            slab_pre = []
            s = wpool.tile([P, NT * P], FP16, name="wslab")
            nc.sync.dma_start(s[:], wt_d[0, 0])
            slab_pre.append(s)
            for pc in range(1, XP):
                nc.sync.dma_start(
                    A[:, pc * xpc : (pc + 1) * xpc], xt_d[0, :, pc * xpc : (pc + 1) * xpc]
                )
            for ko in (1, 2):
                s = wpool.tile([P, NT * P], FP16, name="wslab")
                nc.sync.dma_start(s[:], wt_d[0, ko])
                slab_pre.append(s)

            # PE warmup during the x/slab load (HAM ramp + queue fill).
            wt_warm = warmpool.tile([P, P + NB], FP16, name="wt_warm")
            nc.gpsimd.memset(wt_warm[:], 0.0)
            wp = wppool.tile([P, NB], F32, name="wp")
            for i in range(WARMUP_MMS_S):
                nc.tensor.matmul(wp[:], wt_warm[:, 0:P], wt_warm[:, P:],
                                 start=(i == 0), stop=(i == WARMUP_MMS_S - 1))

            def filler(j):
                # discardable MM whose rhs is an already-arrived x piece, so
                # the scheduler can't run it before that piece's DMA.
                nc.tensor.matmul(wp[:], wt_warm[:, 0:P],
                                 A[:, j * xpc : j * xpc + NB],
                                 start=True, stop=True)

            # Layer 0: two half-sweeps (slab reloaded per half).
            for h in range(2):
                for ko in range(NT):
                    if h == 0 and ko < 3:
                        slab = slab_pre[ko]
                    else:
                        slab = wpool.tile([P, NT * P], FP16, name="wslab")
                        nc.sync.dma_start(slab[:], wt_d[0, ko])
                    if h == 0 and 3 <= ko <= 10:
                        # stream an x-half1 piece between slab DMAs
                        pc = ko - 3
                        nc.sync.dma_start(
                            A[:, HNB + pc * xpc : HNB + (pc + 1) * xpc],
                            xt_d[1, :, pc * xpc : (pc + 1) * xpc],
                        )
                    pt = ppool.tile([P, NB], F32, name="pt")
                    for d in range(NT):
                        nc.tensor.matmul(
                            pt[:], slab[:, d * P : (d + 1) * P],
                            A[:, h * HNB + d * NB : h * HNB + (d + 1) * NB],
                            start=(d == 0), stop=(d == NT - 1),
                        )
                        if h == 0 and ko == 0 and d % 4 == 3 and d // 4 < KO0_FILLERS:
                            filler(d // 4)
                    if h == 0 and ko == 0:
                        for g in range(POST_KO0_FILLERS):
                            filler(g % XP)
                    nc.scalar.activation(
                        Bt[:, ko * BC + h * NB : ko * BC + (h + 1) * NB], pt[:],
                        mybir.ActivationFunctionType.Relu,
                        bias=bias_sb[:, ko : ko + 1], scale=sc,
                    )

            # Layers 1..4 (merged 1024-col sweeps, slab reused for both halves)
            for l in range(1, L):
                ain = Bt if l % 2 == 1 else A
                aout = A if l % 2 == 1 else Bt
                for ko in range(NT):
                    slab = wpool.tile([P, NT * P], FP16, name="wslab")
                    nc.sync.dma_start(slab[:], wt_d[l, ko])
                    pt = ppool.tile([P, 2 * NB], F32, name="pt")
                    for d in range(NT):
                        for h in range(2):
                            nc.tensor.matmul(
                                pt[:, h * NB : (h + 1) * NB],
                                slab[:, d * P : (d + 1) * P],
                                ain[:, d * BC + h * NB : d * BC + (h + 1) * NB],
                                start=(d == 0), stop=(d == NT - 1),
                            )
                    for h in range(2):
                        nc.scalar.activation(
                            aout[:, ko * BC + h * NB : ko * BC + (h + 1) * NB],
                            pt[:, h * NB : (h + 1) * NB],
                            mybir.ActivationFunctionType.Relu,
                            bias=bias_sb[:, l * NT + ko : l * NT + ko + 1], scale=sc,
                        )

            # Output layer: col-group packed partials per half (the burst
            # stream overlaps ko31's ACT drain); evict each [128,512] f32
            # PSUM tile whole (scalar h0 / vector h1, concurrently) and DMA
            # raw - host sums partitions {32j,32j+1}.
            hout = A if L % 2 == 0 else Bt
            zps = []
            for h in range(2):
                zp = zpool.tile([P, NB], F32, name=f"zp{h}")
                zps.append(zp)
                for t in range(NT):
                    j = t % 4
                    nc.tensor.matmul(
                        zp[32 * j : 32 * j + 2, :], wo_sb[:, t * 2 : (t + 1) * 2],
                        hout[:, t * BC + h * NB : t * BC + (h + 1) * NB],
                        start=(t < 4), stop=(t >= NT - 4),
                        tile_position=(0, 32 * j),
                    )
            for h in range(2):
                zs = zsbpool.tile([P, NB], F32, name=f"zs{h}")
                if h == 0:
                    nc.scalar.copy(zs[:], zps[h][:])
                else:
                    nc.vector.tensor_copy(zs[:], zps[h][:])
                nc.sync.dma_start(z_d[h], zs[:])

    nc.compile()
    return nc


def _build_strass():
    """One-level Strassen over the 4096x4096 layer matmuls (all 5 layers).

    W split 2x2 (2048 blocks), h split k-half x batch-half (quadrants, N
    stays 512 = one PSUM bank).  7 products per layer instead of 8: 1792
    MMs/layer vs 2048 -> ~55us/layer PE savings (~277us total).  The 7
    weight operands (entries {-2..2}, exact fp16) are host-precomputed; the
    5 activation combos are DVE adds (fp16) hidden under PE work, two of
    them stored in-place over the dead B12/B21 quadrants of the input tile.
    M order (M2,M5,M3,M4,M7,M6,M1) matches combo readiness (cb3,cb4,cb7,
    cb6,cb1).  Per ko: 7 PSUM M-tiles -> 8 DVE adds + 4 ACT relu evictions
    reassemble C11/C12/C21/C22 into the output quadrants.
    Activation tiles are quadrant-major: col = q*8192 + kt*512 + c with
    q = (k-half<<1)|batch-half; kt = k-tile index within the half (0..15).
    """
    nc = bacc.Bacc("TRN2", target_bir_lowering=False, debug=False)
    NQ = NT // 2      # 16 k/dout tiles per half
    QW = NQ * NB      # 8192 cols per quadrant
    xt_d = nc.dram_tensor("xt", [P, NT * BC], FP16, kind="ExternalInput").ap()
    wt_d = nc.dram_tensor("wt", [L, 7, NQ, P, NQ * P], FP16, kind="ExternalInput").ap()
    bias_d = nc.dram_tensor("biasd", [P, L * NT], F32, kind="ExternalInput").ap()
    wo_d = nc.dram_tensor("wo", [P, NT * 2], FP16, kind="ExternalInput").ap()
    z_d = nc.dram_tensor("z", [2, P, NB], F32, kind="ExternalOutput").ap()
    sc = float(2.0 ** (-FP16_SHIFT))
    # rhs operand per M exec slot: quadrant index of ain (in-place combos
    # land in q1/q2) or a dedicated combo tile (None here, filled below)
    #   e0=M2:q0(B11) e1=M5:q3(B22) e2=M3:cb3 e3=M4:cb4 e4=M7:cb7
    #   e5=M6:q2(<-cb6) e6=M1:q1(<-cb1)

    with tile.TileContext(nc) as tc:
        with (
            tc.tile_pool(name="acts", bufs=1) as acts,
            tc.tile_pool(name="combos", bufs=1) as cbpool,
            tc.tile_pool(name="wpool", bufs=4) as wpool,
            tc.tile_pool(name="const", bufs=1) as cpool,
            tc.tile_pool(name="warm", bufs=1) as warmpool,
            tc.tile_pool(name="ctmp", bufs=6) as ctpool,
            tc.tile_pool(name="mps", bufs=8, space="PSUM") as mpool,
        ):
            A = acts.tile([P, NT * BC], FP16, name="A")
            Bt = acts.tile([P, NT * BC], FP16, name="Bt")
            cb3 = cbpool.tile([P, QW], FP16, name="cb3")
            cb4 = cbpool.tile([P, QW], FP16, name="cb4")
            cb7 = cbpool.tile([P, QW], FP16, name="cb7")
            bias_sb = cpool.tile([P, L * NT], F32, name="bias_sb")
            wo_sb = cpool.tile([P, NT * 2], FP16, name="wo_sb")

            nc.scalar.dma_start(bias_sb[:], bias_d[:])
            nc.scalar.dma_start(wo_sb[:], wo_d[:])

            # startup: x quadrants (1MB pieces, 8KB lines) interleaved with
            # the first five layer-0 G-slabs on the sync ring, in M order.
            pre_keys = [(0, 0), (1, 0), (2, 0), (3, 0)]
            slab_pre = {}
            pre_it = iter(pre_keys)

            def preslab():
                k = next(pre_it, None)
                if k is not None:
                    s = wpool.tile([P, NQ * P], FP16, name="wslab")
                    nc.sync.dma_start(s[:], wt_d[0, k[0], k[1]])
                    slab_pre[k] = s

            def xpiece(q, i):
                c0 = q * QW + i * (QW // 4)
                nc.sync.dma_start(A[:, c0 : c0 + QW // 4], xt_d[:, c0 : c0 + QW // 4])

            xpiece(0, 0); preslab()          # B11 p0, G(M2,k0)
            xpiece(0, 1); xpiece(0, 2); xpiece(0, 3)
            preslab()                        # G(M5,k0)
            for i in range(4):
                xpiece(3, i)                 # B22
            preslab()                        # G(M3,k0)
            for i in range(4):
                xpiece(1, i)                 # B12
            preslab()                        # G(M4,k0)
            for i in range(4):
                xpiece(2, i)                 # B21

            wt_warm = warmpool.tile([P, P + NB], FP16, name="wt_warm")
            nc.gpsimd.memset(wt_warm[:], 0.0)
            wp = mpool.tile([P, NB], F32, name="mt")
            for i in range(STRASS_WARMUP):
                nc.tensor.matmul(wp[:], wt_warm[:, 0:P], wt_warm[:, P:],
                                 start=(i == 0), stop=(i == STRASS_WARMUP - 1))
            # zp tiles pinned early (eager-drain keeps M pressure low) so
            # the output bursts never wait on the last ko's eviction chain;
            # they double as the filler target (unused until the end).
            zps = [mpool.tile([P, NB], F32, name="mt") for _ in range(2)]

            def filler(n):
                for _ in range(n):
                    nc.tensor.matmul(zps[0][:], wt_warm[:, 0:P], A[:, 0:NB],
                                     start=True, stop=True)

            def quad(t, q):
                return t[:, q * QW : (q + 1) * QW]

            for l in range(L):
                ain = A if l % 2 == 0 else Bt
                aout = Bt if l % 2 == 0 else A
                # combos (chunked x4 for finer deps); order matters: q2 is
                # read by cb4/cb7 before cb6 overwrites it, q1 by cb3/cb6
                # before cb1 overwrites it.
                CH4 = QW // 4
                for c in range(4):
                    s_ = slice(c * CH4, (c + 1) * CH4)
                    nc.vector.tensor_sub(cb3[:, s_], quad(ain, 1)[:, s_], quad(ain, 3)[:, s_])
                for c in range(4):
                    s_ = slice(c * CH4, (c + 1) * CH4)
                    nc.vector.tensor_sub(cb4[:, s_], quad(ain, 2)[:, s_], quad(ain, 0)[:, s_])
                for c in range(4):
                    s_ = slice(c * CH4, (c + 1) * CH4)
                    nc.vector.tensor_add(cb7[:, s_], quad(ain, 2)[:, s_], quad(ain, 3)[:, s_])
                for c in range(4):
                    s_ = slice(c * CH4, (c + 1) * CH4)
                    nc.vector.tensor_add(quad(ain, 2)[:, s_], quad(ain, 0)[:, s_], quad(ain, 1)[:, s_])
                for c in range(4):
                    s_ = slice(c * CH4, (c + 1) * CH4)
                    nc.vector.tensor_add(quad(ain, 1)[:, s_], quad(ain, 0)[:, s_], quad(ain, 3)[:, s_])
                rhs_ops = [quad(ain, 0), quad(ain, 3), cb3[:], cb4[:],
                           cb7[:], quad(ain, 2), quad(ain, 1)]
                for ko in range(NQ):
                    # Eager-drain: each M's PSUM is consumed into SBUF
                    # chains right after its 16 MMs (DVE tensor_tensor
                    # allows at most one PSUM operand), so only ~1 M bank
                    # is live at a time and the pinned zp tiles fit.
                    bc1 = bias_sb[:, l * NT + ko : l * NT + ko + 1]
                    bc2 = bias_sb[:, l * NT + NQ + ko : l * NT + NQ + ko + 1]

                    def ct():
                        return ctpool.tile([P, NB], F32, name="ct")

                    def act(q, src, bias):
                        nc.scalar.activation(
                            aout[:, q * QW + ko * NB : q * QW + (ko + 1) * NB],
                            src[:], mybir.ActivationFunctionType.Relu,
                            bias=bias, scale=sc)

                    st = {}
                    for e in range(7):
                        if l == 0:
                            # bridge the PE over DMA/combo waits ahead of
                            # this M-block (q1/q2/slabs still streaming in)
                            filler(STRASS_FILLERS.get((ko, e), 0))
                        if l == 0 and (e, ko) in slab_pre:
                            slab = slab_pre[(e, ko)]
                        else:
                            slab = wpool.tile([P, NQ * P], FP16, name="wslab")
                            nc.sync.dma_start(slab[:], wt_d[l, e, ko])
                        mt = mpool.tile([P, NB], F32, name="mt")
                        for j in range(NQ):
                            nc.tensor.matmul(
                                mt[:], slab[:, j * P : (j + 1) * P],
                                rhs_ops[e][:, j * NB : (j + 1) * NB],
                                start=(j == 0), stop=(j == NQ - 1),
                            )
                        if e == 0:      # m2
                            st["p1"] = ct()
                            nc.vector.tensor_copy(st["p1"][:], mt[:])
                        elif e == 1:    # m5
                            st["p3"] = ct()
                            nc.vector.tensor_copy(st["p3"][:], mt[:])
                        elif e == 2:    # m3
                            q12 = ct()
                            nc.vector.tensor_add(q12[:], st["p3"][:], mt[:])
                            act(1, q12, bc1)               # C12 = M3+M5
                            st["c22"] = ct()
                            nc.vector.tensor_sub(st["c22"][:], mt[:], st["p1"][:])
                        elif e == 3:    # m4
                            q21 = ct()
                            nc.vector.tensor_add(q21[:], st["p1"][:], mt[:])
                            act(2, q21, bc2)               # C21 = M2+M4
                            st["c11"] = ct()
                            nc.vector.tensor_sub(st["c11"][:], mt[:], st["p3"][:])
                        elif e == 4:    # m7
                            n11 = ct()
                            nc.vector.tensor_add(n11[:], st["c11"][:], mt[:])
                            st["c11"] = n11
                        elif e == 5:    # m6
                            n22 = ct()
                            nc.vector.tensor_add(n22[:], st["c22"][:], mt[:])
                            st["c22"] = n22
                        else:           # e == 6: m1
                            q11 = ct()
                            nc.vector.tensor_add(q11[:], st["c11"][:], mt[:])
                            act(0, q11, bc1)               # C11 = M4-M5+M7+M1
                            q22 = ct()
                            nc.vector.tensor_add(q22[:], st["c22"][:], mt[:])
                            act(3, q22, bc2)               # C22 = M3-M2+M6+M1

            # output layer: col-group packed, quadrant-aware rhs.  Tiles
            # t=15/31 read ko15's C-evictions (the very last ACTs) - issue
            # them LAST so the other 56 burst MMs overlap the eviction
            # chain instead of stalling behind t=15 in the PE FIFO.
            # start/stop are per (h, col-group): first/last issued in group.
            hout = A if L % 2 == 0 else Bt
            t_early = [t for t in range(NT) if t % NQ != NQ - 1]
            burst_list = [(h, t) for h in range(2) for t in t_early]
            burst_list += [(0, 15), (1, 31), (1, 15), (0, 31)]
            seen = {}
            for h, t in burst_list:
                j = t % 4
                q = (0 if t < NQ else 2) + h
                c0 = q * QW + (t % NQ) * NB
                k = (h, j)
                seen[k] = seen.get(k, 0) + 1
                nc.tensor.matmul(
                    zps[h][32 * j : 32 * j + 2, :], wo_sb[:, t * 2 : (t + 1) * 2],
                    hout[:, c0 : c0 + NB],
                    start=(seen[k] == 1), stop=(seen[k] == 8),
                    tile_position=(0, 32 * j),
                )
            for h in range(2):
                zs = ctpool.tile([P, NB], F32, name="ct")
                if h == 0:
                    nc.scalar.copy(zs[:], zps[h][:])
                else:
                    nc.vector.tensor_copy(zs[:], zps[h][:])
                nc.sync.dma_start(z_d[h], zs[:])

    nc.compile()
    return nc


def _build_hybrid():
    """fp16 hi + fp8e4m3 lo (DoubleRow) with 2^-6/layer activation rescale.

    h = hi + lo/LOSC; hi pass: 32 fp16 matmuls; lo pass: 16 fp8 DoubleRow
    matmuls (2 k-tiles each) into a separate PSUM bank, combined at evict.
    """
    FP8 = mybir.dt.float8e4
    nc = bacc.Bacc("TRN2", target_bir_lowering=False, debug=False)
    xth_d = nc.dram_tensor("xth", [CH, P, NT * NB], FP16, kind="ExternalInput").ap()
    xtl_d = nc.dram_tensor("xtl", [CH, P, NT * NB], FP8, kind="ExternalInput").ap()
    wt16_d = nc.dram_tensor("wt16", [L, NT, P, NT * P], FP16, kind="ExternalInput").ap()
    wt8_d = nc.dram_tensor("wt8", [L, NT, P, NT * P], FP8, kind="ExternalInput").ap()
    bias_d = nc.dram_tensor("biasd", [P, L * NT], F32, kind="ExternalInput").ap()
    wo16_d = nc.dram_tensor("wo16", [P, NT * 2], FP16, kind="ExternalInput").ap()
    wo8_d = nc.dram_tensor("wo8", [P, NT * 2], FP8, kind="ExternalInput").ap()
    z_d = nc.dram_tensor("z", [2, BC], F32, kind="ExternalOutput").ap()
    sc = float(2.0 ** (-FP16_SHIFT))

    with tile.TileContext(nc) as tc:
        with (
            tc.tile_pool(name="acts", bufs=1) as acts,
            tc.tile_pool(name="w16pool", bufs=3) as w16pool,
            tc.tile_pool(name="w8pool", bufs=3) as w8pool,
            tc.tile_pool(name="tmp", bufs=3) as tpool,
            tc.tile_pool(name="const", bufs=1) as cpool,
            tc.tile_pool(name="psumh", bufs=3, space="PSUM") as pph,
            tc.tile_pool(name="psuml", bufs=3, space="PSUM") as ppl,
            tc.tile_pool(name="zpsum", bufs=1, space="PSUM") as zppool,
            tc.tile_pool(name="zsb", bufs=2) as zsbpool,
        ):
            A_hi = acts.tile([P, NT * NB], FP16, name="A_hi")
            A_lo = acts.tile([P, NT * NB], FP8, name="A_lo")
            B_hi = acts.tile([P, NT * NB], FP16, name="B_hi")
            B_lo = acts.tile([P, NT * NB], FP8, name="B_lo")
            bias_sb = cpool.tile([P, L * NT], F32, name="bias_sb")
            wo16_sb = cpool.tile([P, NT * 2], FP16, name="wo16_sb")
            wo8_sb = cpool.tile([P, NT * 2], FP8, name="wo8_sb")
            nc.sync.dma_start(bias_sb[:], bias_d[:])
            nc.sync.dma_start(wo16_sb[:], wo16_d[:])
            nc.sync.dma_start(wo8_sb[:], wo8_d[:])

            for c in range(CH):
                nc.sync.dma_start(A_hi[:], xth_d[c])
                nc.sync.dma_start(A_lo[:], xtl_d[c])
                for l in range(L):
                    ain_hi, ain_lo = (A_hi, A_lo) if l % 2 == 0 else (B_hi, B_lo)
                    aout_hi, aout_lo = (B_hi, B_lo) if l % 2 == 0 else (A_hi, A_lo)
                    for ko in range(NT):
                        slab16 = w16pool.tile([P, NT * P], FP16, name="w16slab")
                        nc.sync.dma_start(slab16[:], wt16_d[l, ko])
                        slab8 = w8pool.tile([P, NT * P], FP8, name="w8slab")
                        nc.sync.dma_start(slab8[:], wt8_d[l, ko])
                        # NOTE: batched ordering (all fp16, then all DR) measures
                        # faster than hi,hi,lo interleave (219.4 vs 224.2 ns/MM):
                        # alternating Normal/DoubleRow perf modes thrashes the
                        # PE weight path more than the DR LDWEIGHTS costs.
                        pt = pph.tile([P, NB], F32, name="pt")
                        for d in range(NT):
                            nc.tensor.matmul(
                                pt[:], slab16[:, d * P : (d + 1) * P],
                                ain_hi[:, d * NB : (d + 1) * NB],
                                start=(d == 0), stop=(d == NT - 1),
                            )
                        plo = ppl.tile([P, NB], F32, name="plo")
                        for m in range(NT // 2):
                            lhsT = slab8[:, 2 * m * P : (2 * m + 2) * P].rearrange(
                                "p (j c) -> p j c", j=2
                            )
                            rhs = ain_lo[
                                :, 2 * m * NB : (2 * m + 2) * NB
                            ].rearrange("p (j b) -> p j b", j=2)
                            nc.tensor.matmul(
                                plo[:], lhsT, rhs,
                                start=(m == 0), stop=(m == NT // 2 - 1),
                                perf_mode=mybir.MatmulPerfMode.DoubleRow,
                            )
                        # combine + relu + re-split (t4s is relu result x LOSC)
                        t1 = tpool.tile([P, NB], F32, name="t1")
                        nc.scalar.mul(t1[:], plo[:], 1.0 / LOSC)
                        t2 = tpool.tile([P, NB], F32, name="t2")
                        nc.vector.tensor_add(t2[:], t1[:], pt[:])
                        t4s = tpool.tile([P, NB], F32, name="t4s")
                        nc.scalar.activation(
                            t4s[:], t2[:], mybir.ActivationFunctionType.Relu,
                            bias=bias_sb[:, l * NT + ko : l * NT + ko + 1],
                            scale=sc * LOSC,
                        )
                        nc.vector.tensor_scalar_mul(
                            aout_hi[:, ko * NB : (ko + 1) * NB], t4s[:], 1.0 / LOSC
                        )
                        nc.vector.scalar_tensor_tensor(
                            aout_lo[:, ko * NB : (ko + 1) * NB],
                            aout_hi[:, ko * NB : (ko + 1) * NB], -LOSC, t4s[:],
                            op0=mybir.AluOpType.mult, op1=mybir.AluOpType.add,
                        )

                hout_hi, hout_lo = (A_hi, A_lo) if L % 2 == 0 else (B_hi, B_lo)
                zph = zppool.tile([2, NB], F32, name="zph")
                for t in range(NT):
                    nc.tensor.matmul(
                        zph[:], wo16_sb[:, t * 2 : (t + 1) * 2],
                        hout_hi[:, t * NB : (t + 1) * NB],
                        start=(t == 0), stop=(t == NT - 1),
                    )
                zpl = zppool.tile([2, NB], F32, name="zpl")
                for t in range(NT):
                    nc.tensor.matmul(
                        zpl[:], wo8_sb[:, t * 2 : (t + 1) * 2],
                        hout_lo[:, t * NB : (t + 1) * NB],
                        start=(t == 0), stop=(t == NT - 1),
                    )
                zs1 = zsbpool.tile([2, NB], F32, name="zs1")
                nc.scalar.mul(zs1[:], zpl[:], 1.0 / LOSC)
                zs = zsbpool.tile([2, NB], F32, name="zs")
                nc.vector.tensor_add(zs[:], zs1[:], zph[:])
                nc.sync.dma_start(z_d[:, c * NB : (c + 1) * NB], zs[:])

    nc.compile()
    return nc


def _build_program(mode):
    if mode not in _BUILD_CACHE:
        _BUILD_CACHE[mode] = {
            "split2": _build_split2, "fp16": _build_fp16, "hybrid": _build_hybrid,
            "fp16m": _build_fp16m, "fp16s": _build_fp16s, "strass": _build_strass,
        }[mode]()
    return _BUILD_CACHE[mode]


def _split_hilo(a32: np.ndarray):
    hi = a32.astype(BF)
    lo = (a32 - hi.astype(np.float32)).astype(BF)
    return hi, lo


def _weights_layout(w: np.ndarray) -> np.ndarray:
    """[dout, din] f32 -> [NT(ko), P(p), NT*P(t*128+c)]."""
    return w.reshape(NT, P, NT, P).transpose(0, 3, 2, 1).reshape(NT, P, NT * P)


def _prepare_inputs(mode, x, scores1_h, scores2_h, bias_h, scores1_o, scores2_o):
    """Host-side: masks, weight/bias/x layouts for the device program.

    Returns a list of per-core in_maps (without only the core-varying xt)."""
    F8 = ml_dtypes.float8_e4m3
    wdt = BF if mode == "split2" else F16
    if mode == "strass":
        NQ, Q = NT // 2, D // 2
        QW = NQ * NB

        def _wl2048(w):
            return w.reshape(NQ, P, NQ, P).transpose(0, 3, 2, 1).reshape(NQ, P, NQ * P)

        wt = np.empty((L, 7, NQ, P, NQ * P), F16)
        for l in range(L):
            W = _subnet_mask(scores1_h[l]) - _subnet_mask(scores2_h[l])
            A11, A12 = W[:Q, :Q], W[:Q, Q:]
            A21, A22 = W[Q:, :Q], W[Q:, Q:]
            # exec order M2,M5,M3,M4,M7,M6,M1 (combos {-2..2} exact in fp16)
            Gs = (A21 + A22, A11 + A12, A11, A22, A12 - A22, A21 - A11, A11 + A22)
            for i, G in enumerate(Gs):
                wt[l, i] = _wl2048(G).astype(F16)
        scales = 2.0 ** (-FP16_SHIFT * np.arange(1, L + 1, dtype=np.float32))
        b2 = 2.0 * bias_h.astype(np.float32) * scales[:, None]
        bias_sb = np.ascontiguousarray(
            b2.reshape(L, NT, P).transpose(2, 0, 1).reshape(P, L * NT)
        )
        wo = _subnet_mask(scores1_o) - _subnet_mask(scores2_o)
        wo_sb = np.ascontiguousarray(
            wo.reshape(2, NT, P).transpose(2, 1, 0).reshape(P, NT * 2)
        ).astype(F16)
        common = {"wt": wt, "biasd": bias_sb, "wo": wo_sb}
        in_maps = []
        for i in range(N_CORES):
            xT = x[i * BC : (i + 1) * BC].T.astype(np.float32)  # [D, BC]
            blocks = [xT[:Q, :NB], xT[:Q, NB:], xT[Q:, :NB], xT[Q:, NB:]]
            xr = np.concatenate(
                [b.reshape(NQ, P, NB).transpose(1, 0, 2).reshape(P, QW)
                 for b in blocks], axis=1,
            ).astype(F16)
            in_maps.append({**common, "xt": np.ascontiguousarray(xr)})
        return in_maps
    if mode == "fp16s":
        wt = np.empty((L, NT, P, NT * P), F16)
        for l in range(L):
            w = _subnet_mask(scores1_h[l]) - _subnet_mask(scores2_h[l])
            wt[l] = _weights_layout(w).astype(F16)
        scales = 2.0 ** (-FP16_SHIFT * np.arange(1, L + 1, dtype=np.float32))
        b2 = 2.0 * bias_h.astype(np.float32) * scales[:, None]
        bias_sb = np.ascontiguousarray(
            b2.reshape(L, NT, P).transpose(2, 0, 1).reshape(P, L * NT)
        )
        wo = _subnet_mask(scores1_o) - _subnet_mask(scores2_o)
        wo_sb = np.ascontiguousarray(
            wo.reshape(2, NT, P).transpose(2, 1, 0).reshape(P, NT * 2)
        ).astype(F16)
        common = {"wt": wt, "biasd": bias_sb, "wo": wo_sb}
        in_maps = []
        for i in range(N_CORES):
            xT = x[i * BC : (i + 1) * BC].T.astype(np.float32)  # [D, BC]
            # half-major layout: [2(half), P, NT*NB], col = d*NB + c
            xr = np.ascontiguousarray(
                xT.reshape(NT, P, 2, NB).transpose(2, 1, 0, 3).reshape(2, P, NT * NB)
            ).astype(F16)
            in_maps.append({**common, "xt": xr})
        return in_maps
    if mode == "fp16m":
        wt = np.empty((L, NT, P, NT * P), F16)
        for l in range(L):
            w = _subnet_mask(scores1_h[l]) - _subnet_mask(scores2_h[l])
            wt[l] = _weights_layout(w).astype(F16)
        scales = 2.0 ** (-FP16_SHIFT * np.arange(1, L + 1, dtype=np.float32))
        b2 = 2.0 * bias_h.astype(np.float32) * scales[:, None]
        bias_sb = np.ascontiguousarray(
            b2.reshape(L, NT, P).transpose(2, 0, 1).reshape(P, L * NT)
        )
        wo = _subnet_mask(scores1_o) - _subnet_mask(scores2_o)
        wo_sb = np.ascontiguousarray(
            wo.reshape(2, NT, P).transpose(2, 1, 0).reshape(P, NT * 2)
        ).astype(F16)
        sel_np = np.zeros((P, 2), F16)
        for j in range(4):
            sel_np[32 * j, 0] = 1.0
            sel_np[32 * j + 1, 1] = 1.0
        common = {"wt": wt, "biasd": bias_sb, "wo": wo_sb, "sel": sel_np}
        in_maps = []
        for i in range(N_CORES):
            xT = x[i * BC : (i + 1) * BC].T.astype(np.float32)  # [D, BC]
            xr = np.ascontiguousarray(
                xT.reshape(NT, P, BC).transpose(1, 0, 2).reshape(P, NT * BC)
            ).astype(F16)
            in_maps.append({**common, "xt": xr})
        return in_maps
    wt = np.empty((L, NT, P, NT * P), wdt)
    wt8 = np.empty((L, NT, P, NT * P), F8) if mode == "hybrid" else None
    for l in range(L):
        w = _subnet_mask(scores1_h[l]) - _subnet_mask(scores2_h[l])  # [dout, din]
        wl = _weights_layout(w)
        wt[l] = wl.astype(wdt)
        if wt8 is not None:
            wt8[l] = wl.astype(F8)

    b2 = 2.0 * bias_h.astype(np.float32)  # [L, D]
    if mode in ("fp16", "hybrid"):
        scales = (2.0 ** (-FP16_SHIFT * np.arange(1, L + 1, dtype=np.float32)))
        b2 = b2 * scales[:, None]
        if mode == "hybrid":
            b2 = b2 * LOSC
    bias_sb = np.ascontiguousarray(
        b2.reshape(L, NT, P).transpose(2, 0, 1).reshape(P, L * NT)
    )

    wo = _subnet_mask(scores1_o) - _subnet_mask(scores2_o)  # [2, D]
    wo_l = np.ascontiguousarray(
        wo.reshape(2, NT, P).transpose(2, 1, 0).reshape(P, NT * 2)
    )
    wo_sb = wo_l.astype(wdt)

    common = {}
    if mode == "hybrid":
        common = {"wt16": wt, "wt8": wt8, "biasd": bias_sb,
                  "wo16": wo_sb, "wo8": wo_l.astype(F8)}
    else:
        common = {"wt": wt, "biasd": bias_sb, "wo": wo_sb}

    in_maps = []
    for i in range(N_CORES):
        xT = x[i * BC : (i + 1) * BC].T.astype(np.float32)  # [D, BC]
        xr = xT.reshape(NT, P, CH, NB).transpose(2, 1, 0, 3).reshape(CH, P, NT * NB)
        xr = np.ascontiguousarray(xr)
        if mode == "split2":
            hi, lo = _split_hilo(xr)
            xt = {"xt": np.ascontiguousarray(np.stack([hi, lo], axis=1))}
        elif mode == "fp16":
            xt = {"xt": xr.astype(F16)}
        else:
            hi = xr.astype(F16)
            lo8 = ((xr - hi.astype(np.float32)) * LOSC).astype(F8)
            xt = {"xth": hi, "xtl": lo8}
        in_maps.append({**common, **xt})
    return in_maps


def kernel(x, scores1_h, scores2_h, bias_h, scores1_o, scores2_o, bias_o, w_last,
           _trace=False, _run_kwargs=None):
    x = np.asarray(x, np.float32)
    scores1_h = np.asarray(scores1_h, np.float32)
    scores2_h = np.asarray(scores2_h, np.float32)
    bias_h = np.asarray(bias_h, np.float32)
    scores1_o = np.asarray(scores1_o, np.float32)
    scores2_o = np.asarray(scores2_o, np.float32)
    bias_o = np.asarray(bias_o, np.float32)
    w_last = np.asarray(w_last, np.float32)

    in_maps = _prepare_inputs(
        MODE, x, scores1_h, scores2_h, bias_h, scores1_o, scores2_o
    )
    nc = _build_program(MODE)
    # Retry guard: very rare transient HW faults have been observed to produce
    # NaN output (z is tiny, so the check is free). Clean runs are bit-identical.
    _zidx = [0, 1, 32, 33, 64, 65, 96, 97]  # fp16s: used partial partitions
    for attempt in range(3):
        res = run_bass_kernel_spmd(
            nc, in_maps, core_ids=list(range(N_CORES)), trace=_trace,
            **(_run_kwargs or {}),
        )
        zs_all = np.stack([res.results[i]["z"] for i in range(N_CORES)])
        if MODE in ("fp16s", "strass"):
            zs_chk = zs_all[:, :, _zidx, :]  # other partitions are junk
        else:
            zs_chk = zs_all
        zbound = 1e11 if MODE == "split2" else 1e4  # z is 2^-30-scaled otherwise
        if np.isfinite(zs_chk).all() and np.abs(zs_chk).max() < zbound:
            break
        print(f"kernel: bad z detected (attempt {attempt}), retrying", file=sys.stderr)

    # host-side tail: relu(z.T + 2*b_o) @ w_last.T  (tiny: [8192, 2] -> [8192, 1])
    zscale = 1.0 if MODE == "split2" else float(2.0 ** (FP16_SHIFT * L))
    # (fp16/hybrid carry h5 scaled by 2^-30; z output is pre-bias, pre-relu)
    y = np.empty((B, 1), np.float32)
    for i in range(N_CORES):
        if MODE in ("fp16s", "strass"):
            zr = res.results[i]["z"]  # [2(half), P, NB] f32, 4 partials/output
            z2 = zr[:, _zidx, :].reshape(2, 4, 2, NB).sum(axis=1)  # [half, o, c]
            z = np.concatenate([z2[0].T, z2[1].T], axis=0)  # [BC, 2]
        else:
            z = res.results[i]["z"].astype(np.float32).T  # [BC, 2]
        h = np.maximum(z * zscale + 2.0 * bias_o[None, :], 0.0).astype(np.float32)
        y[i * BC : (i + 1) * BC] = h @ w_last.T
    if _trace:
        kernel.last_results = res
    return y

